# revision 1
# baseline (speedup 1.0000x reference)
"""NexusNet GNN message-passing kernel for 8 Trainium2 NeuronCores.

Sharding:
  - nexus_up + nexus MLP: sharded by nexus node (M/8 contiguous segs/core);
    edges routed to the core owning their dst segment (host index prep).
    Aggregation via one-hot matmul on PE into PSUM per 128-seg block.
  - n [M,C,FN] (+ per-plane edge-logit b terms) AllGathered to every core.
  - nexus_down: sharded by planar node (N/8 per core, 2 halves/core/plane).
    Per-edge msg = softmax(logit) * n[dst]; logit = a[src] + b[dst] where
    a is a dense per-node dot(x, We) table.  Scatter-mean by src done with
    dma_scatter_add over CSR-slot-ordered edges (unique idx per call).
  - Final 2-layer MLP feature-major on PE; output transposed on host.
"""

import numpy as np

import concourse.bass as bass
import concourse.bacc as bacc
import concourse.mybir as mybir
import concourse.tile as tile

F32 = mybir.dt.float32
F32R = mybir.dt.float32r
I32 = mybir.dt.int32
I16 = mybir.dt.int16
TANH = mybir.ActivationFunctionType.Tanh
EXP = mybir.ActivationFunctionType.Exp
ALU = mybir.AluOpType

CFG_FULL = dict(P=3, N=100000, M=30000, E=200000, C=5, FP=64, FN=32, NC=8)

B_SC = 1024           # edges per down-phase gather/scatter call
NROW = 192            # padded n-row floats (160 n + 15 b + 17 pad)
AROW = 64             # padded a-row floats (5 a + 1 invdeg + pad)
GRP = 4               # up-phase seg blocks per nexus-MLP group
CHW = 512             # stage-C m chunk width


def _ceil(a, b):
    return (a + b - 1) // b


def _wrap16(a):
    # flat idx j -> (partition j%16, col j//16), replicated to 128 partitions
    w = a.reshape(-1, 16).T.copy()
    return np.tile(w, (8, 1))


def host_prep(inputs, cfg):
    P, N, M, E, C, FP, FN, NC = (cfg[k] for k in
                                 ("P", "N", "M", "E", "C", "FP", "FN", "NC"))
    M_LOC = M // NC
    N_LOC = N // NC
    NH = N_LOC // 2                       # nodes per half
    NHP = _ceil(NH, 128) * 128            # padded half (6272)
    NB = _ceil(M_LOC, 128)                # up seg blocks per core
    NTAB = NHP + 128                      # table rows (+trash region)
    TRASH = NTAB - 1

    x = np.ascontiguousarray(np.asarray(inputs["x"], np.float32)
                             .reshape(P, N, C * FP))
    esrc = np.asarray(inputs["edge_src"])
    edst = np.asarray(inputs["edge_dst"])

    # per-core feature-major x slices: [P, 2, C*FP, NH]
    xloc = x.reshape(P, NC, 2, NH, C * FP).transpose(1, 0, 2, 4, 3)
    xloc = np.ascontiguousarray(xloc, np.float32)
    x_flat = x.reshape(P * N, C * FP)

    # ---------------- UP phase indices ----------------
    per_kp = {}
    max_blk_cnt = 0
    for p in range(P):
        order = np.argsort(edst[p], kind="stable")
        ds, ss = edst[p][order], esrc[p][order]
        bounds = np.searchsorted(ds, np.arange(NC + 1) * M_LOC)
        for k in range(NC):
            sl = slice(bounds[k], bounds[k + 1])
            dsl = (ds[sl] - k * M_LOC).astype(np.int64)
            blk = dsl >> 7
            cnt = np.bincount(blk, minlength=NB)
            max_blk_cnt = max(max_blk_cnt, int(cnt.max(initial=0)))
            per_kp[(k, p)] = (dsl, (ss[sl] + p * N).astype(np.int64), blk, cnt)
    K_UP = max(1, _ceil(max_blk_cnt, 128))
    NBK = NB * K_UP

    up_src = np.zeros((NC, P, NBK * 128), np.int32)
    up_dr = np.full((NC, P, NBK * 128), -1.0, np.float32)
    for (k, p), (dsl, sglob, blk, cnt) in per_kp.items():
        starts = np.concatenate(([0], np.cumsum(cnt)))[:-1]
        r = np.arange(len(dsl)) - np.repeat(starts, cnt)
        pos = blk * (K_UP * 128) + r
        up_src[k, p, pos] = sglob
        up_dr[k, p, pos] = dsl - (blk << 7)
    up_src = up_src.reshape(NC, P, NBK, 128).transpose(0, 1, 3, 2).copy()
    up_dr = up_dr.reshape(NC, P, NBK, 128).transpose(0, 1, 3, 2).copy()

    # ---------------- DOWN phase indices ----------------
    down = {}
    slot_cnt_all = []
    for p in range(P):
        order = np.argsort(esrc[p], kind="stable")
        ss, dd = esrc[p][order], edst[p][order]
        bounds = np.searchsorted(ss, np.arange(2 * NC + 1) * NH)
        for j in range(2 * NC):
            k, h = j // 2, j % 2
            sl = slice(bounds[j], bounds[j + 1])
            s_loc = (ss[sl] - j * NH).astype(np.int64)
            d_loc = dd[sl].astype(np.int64)
            deg = np.bincount(s_loc, minlength=NH)
            starts = np.concatenate(([0], np.cumsum(deg)))[:-1]
            rank = np.arange(len(s_loc)) - np.repeat(starts, deg)
            o2 = np.lexsort((s_loc, rank))
            s2, d2, r2 = s_loc[o2], d_loc[o2], rank[o2]
            scnt = (np.bincount(r2) if len(r2) else np.zeros(1, np.int64))
            slot_cnt_all.append(scnt)
            down[(k, p, h)] = (s2, d2, scnt, deg)
    S_MAX = max(len(s) for s in slot_cnt_all)
    gmax = np.zeros(S_MAX, np.int64)
    for s in slot_cnt_all:
        gmax[: len(s)] = np.maximum(gmax[: len(s)], s)
    calls_s = np.array([_ceil(int(g), B_SC) for g in gmax if g > 0])
    call_off = np.concatenate(([0], np.cumsum(calls_s)))
    NCALLS = int(call_off[-1])
    L = NCALLS * B_SC

    dn_dst = np.zeros((NC, 2 * P, 128, L // 16), np.int16)
    dn_srel = np.zeros((NC, 2 * P, 128, L // 16), np.int16)
    dn_scat = np.zeros((NC, 2 * P, 128, L // 16), np.int16)
    degf = np.ones((NC, 2 * P, NTAB), np.float32)
    for (k, p, h), (s2, d2, scnt, deg) in down.items():
        ph = p * 2 + h
        dstA = np.zeros(L, np.int16)
        srelA = np.zeros(L, np.int16)
        scatA = np.full(L, TRASH, np.int16)
        sstart = np.concatenate(([0], np.cumsum(scnt)))[:-1]
        j = np.arange(len(s2)) - np.repeat(sstart, scnt)
        pos = np.repeat(call_off[: len(scnt)] * B_SC, scnt) + j
        dstA[pos] = d2
        srelA[pos] = s2
        scatA[pos] = s2
        dn_dst[k, ph] = _wrap16(dstA)
        dn_srel[k, ph] = _wrap16(srelA)
        dn_scat[k, ph] = _wrap16(scatA)
        degf[k, ph, :NH] = np.maximum(deg, 1).astype(np.float32)
    # deg layout: [128, 2P*(NTAB//128)]: (r, ph*(NTAB//128)+t) = deg[ph][t*128+r]
    degw = (degf.reshape(NC, 2 * P, NTAB // 128, 128)
            .transpose(0, 3, 1, 2).reshape(NC, 128, -1).copy())

    # ---------------- weights ----------------
    g = lambda n: np.asarray(inputs[n], np.float32)
    Wn1, Wn2, We, Wd1, Wd2 = g("Wn1"), g("Wn2"), g("We"), g("Wd1"), g("Wd2")
    bn1, bn2, be, bd1, bd2 = g("bn1"), g("bn2"), g("be"), g("bd1"), g("bd2")

    wn1t = np.stack([Wn1.transpose(2, 0, 1)[p * FP:(p + 1) * FP]
                     .reshape(FP, C * FN) for p in range(P)]).copy()
    wn2t = Wn2.transpose(2, 0, 1).reshape(FN, C * FN).copy()
    # b-term weights: block-diagonal for classes 0..3 (K = 4*FN) and an
    # augmented [FN+1] block for class 4 whose ones-row adds be for all cols.
    went = We[:, :, 0, FP:]                                   # [P, C, FN]
    wentA = np.zeros((4 * FN, C * P), np.float32)
    for c in range(4):
        wentA[c * FN:(c + 1) * FN, c * P:(c + 1) * P] = went[:, c, :].T
    wentB = np.zeros((FN + 1, C * P), np.float32)
    wentB[:FN, 4 * P:] = went[:, 4, :].T
    wentB[FN, :] = be[:, :, 0].T.reshape(-1)
    bn1c = bn1.reshape(C, FN, 1).copy()
    bn2c = bn2.reshape(C, FN, 1).copy()
    we1 = We[:, :, 0, :FP].transpose(0, 2, 1).copy()          # [P, FP, C]
    wd1t = Wd1.transpose(0, 3, 1, 2).reshape(P, FP + FN, C * FP).copy()
    wd2t = Wd2.transpose(0, 1, 3, 2).copy()                   # [P, C, FP, FP]
    bd1c = bd1.reshape(P, C, FP, 1).copy()
    bd2c = bd2.reshape(P, C, FP, 1).copy()
    iota = np.tile(np.arange(128, dtype=np.float32), (128, 1)).copy()
    ident = np.eye(128, dtype=np.float32)

    meta = dict(cfg=cfg, M_LOC=M_LOC, N_LOC=N_LOC, NH=NH, NHP=NHP,
                NB=NB, K_UP=K_UP, NBK=NBK, NTAB=NTAB, TRASH=TRASH,
                NCALLS=NCALLS, L=L, S_MAX=S_MAX)

    shared = dict(x=x_flat, wn1t=wn1t, wn2t=wn2t, wentA=wentA, wentB=wentB,
                  bn1c=bn1c, bn2c=bn2c, we1=we1, wd1t=wd1t, wd2t=wd2t,
                  bd1c=bd1c, bd2c=bd2c, iota=iota, ident=ident)
    in_maps = []
    for k in range(NC):
        m = dict(shared)
        m.update(xloc=xloc[k], up_src=up_src[k], up_dr=up_dr[k],
                 dn_dst=dn_dst[k], dn_srel=dn_srel[k], dn_scat=dn_scat[k],
                 degw=degw[k])
        in_maps.append(m)
    return in_maps, meta


def build_kernel(meta):
    cfg = meta["cfg"]
    P, N, M, E, C, FP, FN, NC = (cfg[k] for k in
                                 ("P", "N", "M", "E", "C", "FP", "FN", "NC"))
    M_LOC, NH, NHP = meta["M_LOC"], meta["NH"], meta["NHP"]
    NMT = NHP // 128
    NB, K_UP, NBK = meta["NB"], meta["K_UP"], meta["NBK"]
    NTAB, NCALLS, L = meta["NTAB"], meta["NCALLS"], meta["L"]
    CF = C * FP
    CN = C * FN
    NBW = FN + C * P           # nbt rows: class-4 n (FN) + b stack (C*P)
    assert C == 5

    nc = bacc.Bacc("TRN2", num_devices=NC)

    def param(name, shape, dt=F32, out=False):
        return nc.declare_dram_parameter(name, list(shape), dt, isOutput=out)

    x_d = param("x", [P * N, CF])
    xloc_d = param("xloc", [P, 2, CF, NH])
    up_src_d = param("up_src", [P, 128, NBK], I32)
    up_dr_d = param("up_dr", [P, 128, NBK])
    dn_dst_d = param("dn_dst", [2 * P, 128, L // 16], I16)
    dn_srel_d = param("dn_srel", [2 * P, 128, L // 16], I16)
    dn_scat_d = param("dn_scat", [2 * P, 128, L // 16], I16)
    degw_d = param("degw", [128, 2 * P * (NTAB // 128)])
    wn1t_d = param("wn1t", [P, FP, CN])
    wn2t_d = param("wn2t", [FN, CN])
    wentA_d = param("wentA", [4 * FN, C * P])
    wentB_d = param("wentB", [FN + 1, C * P])
    bn1c_d = param("bn1c", [C, FN, 1])
    bn2c_d = param("bn2c", [C, FN, 1])
    we1_d = param("we1", [P, FP, C])
    wd1t_d = param("wd1t", [P, FP + FN, C * FP])
    wd2t_d = param("wd2t", [P, C, FP, FP])
    bd1c_d = param("bd1c", [P, C, FP, 1])
    bd2c_d = param("bd2c", [P, C, FP, 1])
    iota_d = param("iota", [128, 128])
    ident_d = param("ident", [128, 128])
    out_d = param("outT", [P, 2, C, FP, NHP], out=True)

    n_loc = nc.dram_tensor("n_loc", [M_LOC, NROW], F32)
    n_full = nc.dram_tensor("n_full", [NC * M_LOC, NROW], F32,
                            addr_space="Shared")
    a_tabs = [nc.dram_tensor(f"a_tab{i}", [NTAB, AROW], F32)
              for i in range(2 * P)]
    s_tabs = [nc.dram_tensor(f"s_tab{i}", [NTAB, NROW], F32)
              for i in range(2 * P)]

    with tile.TileContext(nc) as tc:
        with tc.tile_pool(name="const", bufs=1) as cp:
            iota_t = cp.tile([128, 128], F32R)
            nc.sync.dma_start(out=iota_t[:], in_=iota_d[:].bitcast(F32R))
            ident_t = cp.tile([128, 128], F32)
            nc.sync.dma_start(out=ident_t[:], in_=ident_d[:])
            wn1t_t = [cp.tile([FP, CN], F32R, name=f"wn1t{p}")
                      for p in range(P)]
            wn2t_t = cp.tile([FN, CN], F32R)
            wentA_t = cp.tile([4 * FN, C * P], F32R)
            wentB_t = cp.tile([FN + 1, C * P], F32R)
            nc.sync.dma_start(out=wn2t_t[:], in_=wn2t_d[:].bitcast(F32R))
            nc.sync.dma_start(out=wentA_t[:], in_=wentA_d[:].bitcast(F32R))
            nc.sync.dma_start(out=wentB_t[:], in_=wentB_d[:].bitcast(F32R))
            bn1c_t = [cp.tile([FN, 1], F32, name=f"bn1c{c}") for c in range(C)]
            bn2c_t = [cp.tile([FN, 1], F32, name=f"bn2c{c}") for c in range(C)]
            we1_t = [cp.tile([FP, C], F32, name=f"we1{p}") for p in range(P)]
            wd1t_t = [cp.tile([FP + FN, C * FP], F32R, name=f"wd1t{p}")
                      for p in range(P)]
            wd2t_t = [[cp.tile([FP, FP], F32R, name=f"wd2t{p}_{c}")
                       for c in range(C)] for p in range(P)]
            bd1c_t = [[cp.tile([FP, 1], F32, name=f"bd1c{p}_{c}")
                       for c in range(C)] for p in range(P)]
            bd2c_t = [[cp.tile([FP, 1], F32, name=f"bd2c{p}_{c}")
                       for c in range(C)] for p in range(P)]
            for p in range(P):
                nc.sync.dma_start(out=wn1t_t[p][:], in_=wn1t_d[p].bitcast(F32R))
                nc.sync.dma_start(out=we1_t[p][:], in_=we1_d[p])
                nc.sync.dma_start(out=wd1t_t[p][:], in_=wd1t_d[p].bitcast(F32R))
                for c in range(C):
                    nc.sync.dma_start(out=wd2t_t[p][c][:],
                                      in_=wd2t_d[p, c].bitcast(F32R))
                    nc.sync.dma_start(out=bd1c_t[p][c][:], in_=bd1c_d[p, c])
                    nc.sync.dma_start(out=bd2c_t[p][c][:], in_=bd2c_d[p, c])
            for c in range(C):
                nc.sync.dma_start(out=bn1c_t[c][:], in_=bn1c_d[c])
                nc.sync.dma_start(out=bn2c_t[c][:], in_=bn2c_d[c])
            upsrc_t = [cp.tile([128, NBK], I32, name=f"upsrc{p}")
                       for p in range(P)]
            updr_t = [cp.tile([128, NBK], F32, name=f"updr{p}")
                      for p in range(P)]
            for p in range(P):
                nc.scalar.dma_start(out=upsrc_t[p][:], in_=up_src_d[p])
                nc.scalar.dma_start(out=updr_t[p][:], in_=up_dr_d[p])
            degw_t = cp.tile([128, 2 * P * (NTAB // 128)], F32)
            nc.scalar.dma_start(out=degw_t[:], in_=degw_d[:])

            # zero-init s tables
            zt = cp.tile([128, NROW], F32)
            nc.vector.memset(zt[:], 0.0)
            ones_f = cp.tile([1, GRP * 128], F32)
            nc.vector.memset(ones_f[:], 1.0)
            ones_r = cp.tile([1, GRP * 128], F32R)
            nc.vector.tensor_copy(out=ones_r[:], in_=ones_f[:])
            zeros_r = cp.tile([128, 64], F32R)
            nc.vector.tensor_copy(out=zeros_r[:], in_=zt[:, :64])
            for i in range(2 * P):
                st3 = s_tabs[i].ap().rearrange("(t q) r -> t q r", q=128)
                for t in range(NTAB // 128):
                    nc.sync.dma_start(out=st3[t], in_=zt[:])

            # ======================= UP PHASE =======================
            n_loc_ap = n_loc.ap()
            with tc.tile_pool(name="up_sb", bufs=3) as up, \
                 tc.tile_pool(name="up_sb1", bufs=2) as up1, \
                 tc.tile_pool(name="up_ps", bufs=2, space="PSUM") as upp, \
                 tc.tile_pool(name="up_ps1", bufs=1, space="PSUM") as upp1, \
                 tc.tile_pool(name="mlp_ps", bufs=1, space="PSUM") as mpp:
                for g0 in range(0, NB, GRP):
                    gb = list(range(g0, min(g0 + GRP, NB)))
                    GW = len(gb) * 128
                    # per-plane per-class feature-major up tiles [64, GRP*128]
                    upX = [[up1.tile([FP, GRP * 128], F32R,
                                     name=f"upX{p}_{c}", tag=f"upX{p}_{c}")
                            for c in range(C)] for p in range(P)]
                    for p in range(P):
                        for bi, b in enumerate(gb):
                            pu = upp.tile([128, CF], F32, tag="pu",
                                          space="PSUM")
                            for kk in range(K_UP):
                                col = b * K_UP + kk
                                G = up.tile([128, CF], F32R, tag="G")
                                nc.gpsimd.indirect_dma_start(
                                    out=G[:], out_offset=None,
                                    in_=x_d[:].bitcast(F32R),
                                    in_offset=bass.IndirectOffsetOnAxis(
                                        ap=upsrc_t[p][:, col:col + 1], axis=0))
                                O = up.tile([128, 128], F32R, tag="O")
                                nc.vector.tensor_tensor(
                                    out=O[:],
                                    in0=updr_t[p][:, col:col + 1]
                                        .bitcast(F32R).to_broadcast([128, 128]),
                                    in1=iota_t[:],
                                    op=ALU.is_equal)
                                nc.tensor.matmul(out=pu[:], lhsT=O[:],
                                                 rhs=G[:], start=(kk == 0),
                                                 stop=(kk == K_UP - 1))
                            stg = up.tile([128, CF], F32, tag="stg")
                            nc.scalar.copy(out=stg[:], in_=pu[:])
                            csl = slice(bi * 128, (bi + 1) * 128)
                            for ti in range(3):
                                w = min(128, CF - ti * 128)
                                pt = upp1.tile([128, 128], F32, tag="ptr",
                                               space="PSUM")
                                nc.tensor.transpose(
                                    out=pt[:w, :],
                                    in_=stg[:, ti * 128:ti * 128 + w],
                                    identity=ident_t[:])
                                nc.vector.tensor_copy(
                                    out=upX[p][2 * ti][:, csl],
                                    in_=pt[0:FP, :])
                                if 2 * ti + 1 < C:
                                    nc.vector.tensor_copy(
                                        out=upX[p][2 * ti + 1][:, csl],
                                        in_=pt[FP:2 * FP, :])
                    # ---- nexus MLP over this group ----
                    n1c = [up.tile([FN, GRP * 128], F32R, name=f"n1c{c}",
                                   tag=f"n1c{c}") for c in range(C)]
                    for c in range(C):
                        pn1 = mpp.tile([FN, GRP * 128], F32, tag="pn1",
                                       space="PSUM", bufs=2)
                        for p in range(P):
                            nc.tensor.matmul(
                                out=pn1[:, :GW],
                                lhsT=wn1t_t[p][:, c * FN:(c + 1) * FN],
                                rhs=upX[p][c][:, :GW],
                                start=(p == 0), stop=(p == P - 1))
                        nc.scalar.activation(n1c[c][:, :GW], pn1[:, :GW],
                                             TANH, bias=bn1c_t[c][:])
                    n2s = up.tile([4 * FN, GRP * 128], F32R, tag="n2s")
                    nbt = up.tile([FN + 1, GRP * 128], F32R, tag="nbt")
                    nc.vector.tensor_copy(out=nbt[FN:FN + 1, :],
                                          in_=ones_r[:])
                    for c in range(C):
                        pn2 = mpp.tile([FN, GRP * 128], F32, tag="pn2",
                                       space="PSUM", bufs=2)
                        nc.tensor.matmul(
                            out=pn2[:, :GW],
                            lhsT=wn2t_t[:, c * FN:(c + 1) * FN],
                            rhs=n1c[c][:, :GW], start=True, stop=True)
                        dst = (n2s[c * FN:(c + 1) * FN, :GW] if c < 4
                               else nbt[0:FN, :GW])
                        nc.scalar.activation(dst, pn2[:, :GW],
                                             TANH, bias=bn2c_t[c][:])
                    pbv = mpp.tile([C * P, GRP * 128], F32, tag="misc",
                                   space="PSUM", bufs=1)
                    nc.tensor.matmul(out=pbv[:, :GW], lhsT=wentA_t[:],
                                     rhs=n2s[:, :GW], start=True, stop=False)
                    nc.tensor.matmul(out=pbv[:, :GW], lhsT=wentB_t[:],
                                     rhs=nbt[:, :GW], start=False, stop=True)
                    bt = up.tile([C * P, GRP * 128], F32, tag="bt")
                    nc.vector.tensor_copy(out=bt[:, :GW], in_=pbv[:, :GW])
                    # assemble + store n rows per block
                    for bi, b in enumerate(gb):
                        rows = min(128, M_LOC - b * 128)
                        sl = slice(bi * 128, bi * 128 + 128)
                        tp = mpp.tile([128, 4 * FN + FN + C * P], F32,
                                      tag="misc", space="PSUM", bufs=1)
                        nc.tensor.transpose(
                            out=tp[:, 0:4 * FN],
                            in_=n2s[:, sl].bitcast(F32),
                            identity=ident_t[:])
                        nc.tensor.transpose(
                            out=tp[:, 4 * FN:CN],
                            in_=nbt[0:FN, sl].bitcast(F32),
                            identity=ident_t[:FN, :FN])
                        nc.tensor.transpose(
                            out=tp[:, CN:CN + C * P],
                            in_=bt[:, sl],
                            identity=ident_t[:C * P, :C * P])
                        nrow = up.tile([128, NROW], F32, tag="nrow")
                        nc.vector.tensor_copy(out=nrow[:, 0:CN + C * P],
                                              in_=tp[:])
                        nc.vector.memset(nrow[:, CN + C * P:], 0.0)
                        nc.sync.dma_start(
                            out=n_loc_ap[b * 128:b * 128 + rows, :],
                            in_=nrow[:rows, :])

            # ================= AllGather n =================
            nc.gpsimd.collective_compute(
                "AllGather", ALU.bypass,
                replica_groups=[list(range(NC))],
                ins=[n_loc.ap().opt()], outs=[n_full.ap().opt()])

            # ================= STAGE A: a tables =================
            with tc.tile_pool(name="sa_sb", bufs=2) as sa, \
                 tc.tile_pool(name="sa_ps", bufs=2, space="PSUM") as sap:
                for ph in range(2 * P):
                    p, h = ph // 2, ph % 2
                    for ch0 in range(0, NHP, CHW):
                        cw = min(CHW, NHP - ch0)
                        rw = min(max(NH - ch0, 0), cw)   # real cols
                        xtc = [sa.tile([FP, CHW], F32, name=f"xtc{c}",
                                       tag=f"xtc{c}") for c in range(C)]
                        for c in range(C):
                            if rw < cw:
                                nc.vector.memset(xtc[c][:, rw:cw], 0.0)
                            if rw > 0:
                                nc.sync.dma_start(
                                    out=xtc[c][:, :rw],
                                    in_=xloc_d[p, h, c * FP:(c + 1) * FP,
                                               ch0:ch0 + rw])
                        for j in range(cw // 128):
                            t = ch0 // 128 + j
                            pa = sap.tile([128, C], F32, tag="pa",
                                          space="PSUM")
                            for c in range(C):
                                nc.tensor.matmul(
                                    out=pa[:, c:c + 1],
                                    lhsT=xtc[c][:, j * 128:(j + 1) * 128],
                                    rhs=we1_t[p][:, c:c + 1],
                                    start=True, stop=True)
                            ast = sa.tile([128, AROW], F32, tag="ast")
                            nc.vector.memset(ast[:, C + 1:], 0.0)
                            nc.vector.tensor_copy(out=ast[:, 0:C], in_=pa[:])
                            nc.vector.reciprocal(
                                out=ast[:, C:C + 1],
                                in_=degw_t[:, ph * (NTAB // 128) + t:
                                           ph * (NTAB // 128) + t + 1])
                            nc.sync.dma_start(
                                out=a_tabs[ph].ap()[t * 128:(t + 1) * 128, :],
                                in_=ast[:])
                    for t in range(NMT, NTAB // 128):
                        nc.sync.dma_start(
                            out=a_tabs[ph].ap()[t * 128:(t + 1) * 128, :],
                            in_=zt[:, :AROW])

            # ================= STAGE B: edge stream =================
            NSL = B_SC // 128
            W16 = B_SC // 16
            with tc.tile_pool(name="sb_idx", bufs=1) as ip, \
                 tc.tile_pool(name="sb_sb", bufs=6) as sbp:
                dst_t, srel_t, scat_t = [], [], []
                for ph in range(2 * P):
                    d = ip.tile([128, L // 16], I16, name=f"dt{ph}")
                    nc.scalar.dma_start(out=d[:], in_=dn_dst_d[ph])
                    s = ip.tile([128, L // 16], I16, name=f"srt{ph}")
                    nc.scalar.dma_start(out=s[:], in_=dn_srel_d[ph])
                    sc = ip.tile([128, L // 16], I16, name=f"sct{ph}")
                    nc.scalar.dma_start(out=sc[:], in_=dn_scat_d[ph])
                    dst_t.append(d)
                    srel_t.append(s)
                    scat_t.append(sc)
                for cix in range(NCALLS):
                    for ph in range(2 * P):
                        p = ph // 2
                        isl = slice(cix * W16, (cix + 1) * W16)
                        gn = sbp.tile([128, NSL, NROW], F32, tag="gn")
                        nc.gpsimd.dma_gather(
                            out_ap=gn[:], in_ap=n_full.ap()[:],
                            idxs_ap=dst_t[ph][:, isl],
                            num_idxs=B_SC, num_idxs_reg=B_SC, elem_size=NROW)
                        ga = sbp.tile([128, NSL, AROW], F32, tag="ga")
                        nc.gpsimd.dma_gather(
                            out_ap=ga[:], in_ap=a_tabs[ph].ap()[:],
                            idxs_ap=srel_t[ph][:, isl],
                            num_idxs=B_SC, num_idxs_reg=B_SC, elem_size=AROW)
                        lg = sbp.tile([128, NSL, C], F32, tag="lg")
                        nc.vector.tensor_tensor(
                            out=lg[:], in0=ga[:, :, 0:C],
                            in1=gn[:, :, CN + p:CN + p + (C - 1) * P + 1:P],
                            op=ALU.add)
                        mx = sbp.tile([128, NSL], F32, tag="mx")
                        nc.vector.tensor_reduce(out=mx[:], in_=lg[:],
                                                axis=mybir.AxisListType.X,
                                                op=ALU.max)
                        nc.vector.tensor_tensor(
                            out=lg[:], in0=lg[:],
                            in1=mx[:].to_broadcast([128, NSL, C]),
                            op=ALU.subtract)
                        ex = sbp.tile([128, NSL, C], F32, tag="ex")
                        nc.scalar.activation(ex[:], lg[:], EXP)
                        sm = sbp.tile([128, NSL], F32, tag="sm")
                        nc.vector.tensor_reduce(out=sm[:], in_=ex[:],
                                                axis=mybir.AxisListType.X,
                                                op=ALU.add)
                        nc.vector.reciprocal(out=sm[:], in_=sm[:])
                        nc.vector.tensor_tensor(out=sm[:], in0=sm[:],
                                                in1=ga[:, :, C],
                                                op=ALU.mult)
                        nc.vector.tensor_tensor(
                            out=ex[:], in0=ex[:],
                            in1=sm[:].to_broadcast([128, NSL, C]),
                            op=ALU.mult)
                        msg = sbp.tile([128, NSL, NROW], F32, tag="msg")
                        nc.vector.memset(msg[:, :, CN:], 0.0)
                        nc.vector.tensor_tensor(
                            out=msg[:, :, 0:CN].rearrange(
                                "a b (c f) -> a b c f", f=FN),
                            in0=gn[:, :, 0:CN].rearrange(
                                "a b (c f) -> a b c f", f=FN),
                            in1=ex[:].to_broadcast([128, NSL, C, FN]),
                            op=ALU.mult)
                        nc.gpsimd.dma_scatter_add(
                            out_ap=s_tabs[ph].ap()[:], in_ap=msg[:],
                            idxs_ap=scat_t[ph][:, isl],
                            num_idxs=B_SC, num_idxs_reg=B_SC, elem_size=NROW)

            # ================= STAGE C: down MLP =================
            with tc.tile_pool(name="sc_sb", bufs=3) as scb, \
                 tc.tile_pool(name="sc_ft", bufs=1) as ftp, \
                 tc.tile_pool(name="sc_ps", bufs=2, space="PSUM") as scp:
                for ph in range(2 * P):
                    p, h = ph // 2, ph % 2
                    ft = [ftp.tile([FP + FN, NHP], F32R, name=f"ft{c}",
                                   tag=f"ft{c}") for c in range(C)]
                    for c in range(C):
                        if NHP > NH:
                            nc.vector.tensor_copy(
                                out=ft[c][:, NH:],
                                in_=zeros_r[:FP + FN, :NHP - NH])
                        nc.sync.dma_start(
                            out=ft[c][0:FP, :NH],
                            in_=xloc_d[p, h, c * FP:(c + 1) * FP, :]
                                .bitcast(F32R))
                    for t in range(NMT):
                        st = scb.tile([128, NROW], F32, tag="st")
                        nc.sync.dma_start(
                            out=st[:],
                            in_=s_tabs[ph].ap()[t * 128:(t + 1) * 128, :])
                        t1 = scp.tile([128, 128], F32, tag="st1", space="PSUM")
                        nc.tensor.transpose(out=t1[:, 0:4 * FN],
                                            in_=st[:, 0:4 * FN],
                                            identity=ident_t[:])
                        for c in range(4):
                            nc.vector.tensor_copy(
                                out=ft[c][FP:FP + FN, t * 128:(t + 1) * 128],
                                in_=t1[c * FN:(c + 1) * FN, :])
                        t2 = scp.tile([FN, 128], F32, tag="st2", space="PSUM")
                        nc.tensor.transpose(out=t2[:],
                                            in_=st[:, 4 * FN:CN],
                                            identity=ident_t[:])
                        nc.vector.tensor_copy(
                            out=ft[4][FP:FP + FN, t * 128:(t + 1) * 128],
                            in_=t2[:])
                    for ch0 in range(0, NHP, CHW):
                        cw = min(CHW, NHP - ch0)
                        csl = slice(ch0, ch0 + cw)
                        for c in range(C):
                            hps = scp.tile([FP, CHW], F32, tag="hps",
                                           space="PSUM")
                            nc.tensor.matmul(
                                out=hps[:, :cw],
                                lhsT=wd1t_t[p][:, c * FP:(c + 1) * FP],
                                rhs=ft[c][:, csl], start=True, stop=True)
                            ht = scb.tile([FP, CHW], F32R, tag="ht")
                            nc.scalar.activation(ht[:, :cw], hps[:, :cw],
                                                 TANH, bias=bd1c_t[p][c][:])
                            ops_ = scp.tile([FP, CHW], F32, tag="ops",
                                            space="PSUM")
                            nc.tensor.matmul(
                                out=ops_[:, :cw], lhsT=wd2t_t[p][c][:],
                                rhs=ht[:, :cw], start=True, stop=True)
                            ot = scb.tile([FP, CHW], F32, tag="ot")
                            nc.scalar.activation(ot[:, :cw], ops_[:, :cw],
                                                 TANH, bias=bd2c_t[p][c][:])
                            nc.sync.dma_start(
                                out=out_d[p, h, c, :, csl],
                                in_=ot[:, :cw])

    nc.compile()
    return nc


_CACHE = {}


def _get_compiled(inputs, cfg):
    in_maps, meta = host_prep(inputs, cfg)
    key = (meta["K_UP"], meta["NCALLS"], meta["S_MAX"],
           tuple(sorted(cfg.items())))
    if key not in _CACHE:
        _CACHE[key] = build_kernel(meta)
    return _CACHE[key], in_maps, meta


def assemble_output(results, meta):
    cfg = meta["cfg"]
    P, N, C, FP, NC = (cfg[k] for k in ("P", "N", "C", "FP", "NC"))
    NH = meta["NH"]
    # results[k]["outT"]: [P, 2, C, FP, NHP]
    arr = np.stack([np.asarray(results[k]["outT"])[:, :, :, :, :NH]
                    for k in range(NC)])
    # [NC, P, 2, C, FP, NH] -> [P, NC, 2, NH, C, FP]
    out = arr.transpose(1, 0, 2, 5, 3, 4).reshape(P, N, C, FP)
    return np.ascontiguousarray(out)


def kernel(**inputs):
    from concourse.bass_utils import run_bass_kernel_spmd
    cfg = CFG_FULL
    nc, in_maps, meta = _get_compiled(inputs, cfg)
    res = run_bass_kernel_spmd(nc, in_maps, list(range(cfg["NC"])))
    return assemble_output(res.results, meta)



# revision 3
# speedup vs baseline: 1.3153x; 1.3153x over previous
"""NexusNet GNN message-passing kernel for 8 Trainium2 NeuronCores.

Sharding:
  - nexus_up + nexus MLP: sharded by nexus node (M/8 contiguous segs/core);
    edges routed to the core owning their dst segment (host index prep).
    Aggregation via one-hot matmul on PE into PSUM per 128-seg block.
  - n [M,C,FN] (+ per-plane edge-logit b terms) AllGathered to every core.
  - nexus_down: sharded by planar node (N/8 per core, 2 halves/core/plane).
    Per-edge msg = softmax(logit) * n[dst]; logit = a[src] + b[dst] where
    a is a dense per-node dot(x, We) table.  Scatter-mean by src done with
    dma_scatter_add over CSR-slot-ordered edges (unique idx per call).
  - Final 2-layer MLP feature-major on PE; output transposed on host.
"""

import numpy as np

import concourse.bass as bass
import concourse.bacc as bacc
import concourse.mybir as mybir
import concourse.tile as tile

F32 = mybir.dt.float32
F32R = mybir.dt.float32r
I32 = mybir.dt.int32
I16 = mybir.dt.int16
TANH = mybir.ActivationFunctionType.Tanh
EXP = mybir.ActivationFunctionType.Exp
ALU = mybir.AluOpType

CFG_FULL = dict(P=3, N=100000, M=30000, E=200000, C=5, FP=64, FN=32, NC=8)

B_SC = 1024           # edges per down-phase gather/scatter call
NROW = 192            # padded n-row floats (160 n + 15 b + 17 pad)
AROW = 64             # padded a-row floats (5 a + 1 invdeg + pad)
GRP = 4               # up-phase seg blocks per nexus-MLP group
CHW = 512             # stage-C m chunk width


def _ceil(a, b):
    return (a + b - 1) // b


def _wrap16(a):
    # flat idx j -> (partition j%16, col j//16), replicated to 128 partitions
    w = a.reshape(-1, 16).T.copy()
    return np.tile(w, (8, 1))


def host_prep(inputs, cfg):
    P, N, M, E, C, FP, FN, NC = (cfg[k] for k in
                                 ("P", "N", "M", "E", "C", "FP", "FN", "NC"))
    M_LOC = M // NC
    N_LOC = N // NC
    NH = N_LOC // 2                       # nodes per half
    NHP = _ceil(NH, 128) * 128            # padded half (6272)
    NB = _ceil(M_LOC, 128)                # up seg blocks per core
    NTAB = NHP + 128                      # table rows (+trash region)
    TRASH = NTAB - 1

    x = np.ascontiguousarray(np.asarray(inputs["x"], np.float32)
                             .reshape(P, N, C * FP))
    esrc = np.asarray(inputs["edge_src"])
    edst = np.asarray(inputs["edge_dst"])

    # per-core feature-major x slices: [P, 2, C*FP, NH]
    xloc = x.reshape(P, NC, 2, NH, C * FP).transpose(1, 0, 2, 4, 3)
    xloc = np.ascontiguousarray(xloc, np.float32)
    x_flat = x.reshape(P * N, C * FP)

    # ---------------- UP phase indices ----------------
    per_kp = {}
    max_blk_cnt = 0
    for p in range(P):
        order = np.argsort(edst[p], kind="stable")
        ds, ss = edst[p][order], esrc[p][order]
        bounds = np.searchsorted(ds, np.arange(NC + 1) * M_LOC)
        for k in range(NC):
            sl = slice(bounds[k], bounds[k + 1])
            dsl = (ds[sl] - k * M_LOC).astype(np.int64)
            blk = dsl >> 7
            cnt = np.bincount(blk, minlength=NB)
            max_blk_cnt = max(max_blk_cnt, int(cnt.max(initial=0)))
            per_kp[(k, p)] = (dsl, (ss[sl] + p * N).astype(np.int64), blk, cnt)
    K_UP = max(1, _ceil(max_blk_cnt, 128))
    NBK = NB * K_UP

    up_src = np.zeros((NC, P, NBK * 128), np.int32)
    up_dr = np.full((NC, P, NBK * 128), -1.0, np.float32)
    for (k, p), (dsl, sglob, blk, cnt) in per_kp.items():
        starts = np.concatenate(([0], np.cumsum(cnt)))[:-1]
        r = np.arange(len(dsl)) - np.repeat(starts, cnt)
        pos = blk * (K_UP * 128) + r
        up_src[k, p, pos] = sglob
        up_dr[k, p, pos] = dsl - (blk << 7)
    up_src = up_src.reshape(NC, P, NBK, 128).transpose(0, 1, 3, 2).copy()
    up_dr = up_dr.reshape(NC, P, NBK, 128).transpose(0, 1, 3, 2).copy()

    # ---------------- DOWN phase indices ----------------
    down = {}
    slot_cnt_all = []
    for p in range(P):
        order = np.argsort(esrc[p], kind="stable")
        ss, dd = esrc[p][order], edst[p][order]
        bounds = np.searchsorted(ss, np.arange(2 * NC + 1) * NH)
        for j in range(2 * NC):
            k, h = j // 2, j % 2
            sl = slice(bounds[j], bounds[j + 1])
            s_loc = (ss[sl] - j * NH).astype(np.int64)
            d_loc = dd[sl].astype(np.int64)
            deg = np.bincount(s_loc, minlength=NH)
            starts = np.concatenate(([0], np.cumsum(deg)))[:-1]
            rank = np.arange(len(s_loc)) - np.repeat(starts, deg)
            o2 = np.lexsort((s_loc, rank))
            s2, d2, r2 = s_loc[o2], d_loc[o2], rank[o2]
            scnt = (np.bincount(r2) if len(r2) else np.zeros(1, np.int64))
            slot_cnt_all.append(scnt)
            down[(k, p, h)] = (s2, d2, scnt, deg)
    S_MAX = max(len(s) for s in slot_cnt_all)
    gmax = np.zeros(S_MAX, np.int64)
    for s in slot_cnt_all:
        gmax[: len(s)] = np.maximum(gmax[: len(s)], s)
    calls_s = np.array([_ceil(int(g), B_SC) for g in gmax if g > 0])
    call_off = np.concatenate(([0], np.cumsum(calls_s)))
    NCALLS = int(call_off[-1])
    L = NCALLS * B_SC

    dn_dst = np.zeros((NC, 2 * P, 128, L // 16), np.int16)
    dn_srel = np.zeros((NC, 2 * P, 128, L // 16), np.int16)
    dn_scat = np.zeros((NC, 2 * P, 128, L // 16), np.int16)
    degf = np.ones((NC, 2 * P, NTAB), np.float32)
    for (k, p, h), (s2, d2, scnt, deg) in down.items():
        ph = p * 2 + h
        dstA = np.zeros(L, np.int16)
        srelA = np.zeros(L, np.int16)
        scatA = np.full(L, TRASH, np.int16)
        sstart = np.concatenate(([0], np.cumsum(scnt)))[:-1]
        j = np.arange(len(s2)) - np.repeat(sstart, scnt)
        pos = np.repeat(call_off[: len(scnt)] * B_SC, scnt) + j
        dstA[pos] = d2
        srelA[pos] = s2
        scatA[pos] = s2
        dn_dst[k, ph] = _wrap16(dstA)
        dn_srel[k, ph] = _wrap16(srelA)
        dn_scat[k, ph] = _wrap16(scatA)
        degf[k, ph, :NH] = np.maximum(deg, 1).astype(np.float32)
    # deg layout: [128, 2P*(NTAB//128)]: (r, ph*(NTAB//128)+t) = deg[ph][t*128+r]
    degw = (degf.reshape(NC, 2 * P, NTAB // 128, 128)
            .transpose(0, 3, 1, 2).reshape(NC, 128, -1).copy())

    # ---------------- weights ----------------
    g = lambda n: np.asarray(inputs[n], np.float32)
    Wn1, Wn2, We, Wd1, Wd2 = g("Wn1"), g("Wn2"), g("We"), g("Wd1"), g("Wd2")
    bn1, bn2, be, bd1, bd2 = g("bn1"), g("bn2"), g("be"), g("bd1"), g("bd2")

    wn1t = np.stack([Wn1.transpose(2, 0, 1)[p * FP:(p + 1) * FP]
                     .reshape(FP, C * FN) for p in range(P)]).copy()
    wn2t = Wn2.transpose(2, 0, 1).reshape(FN, C * FN).copy()
    # b-term weights: block-diagonal for classes 0..3 (K = 4*FN) and an
    # augmented [FN+1] block for class 4 whose ones-row adds be for all cols.
    went = We[:, :, 0, FP:]                                   # [P, C, FN]
    wentA = np.zeros((4 * FN, C * P), np.float32)
    for c in range(4):
        wentA[c * FN:(c + 1) * FN, c * P:(c + 1) * P] = went[:, c, :].T
    wentB = np.zeros((FN + 1, C * P), np.float32)
    wentB[:FN, 4 * P:] = went[:, 4, :].T
    wentB[FN, :] = be[:, :, 0].T.reshape(-1)
    bn1c = bn1.reshape(C, FN, 1).copy()
    bn2c = bn2.reshape(C, FN, 1).copy()
    we1 = We[:, :, 0, :FP].transpose(0, 2, 1).copy()          # [P, FP, C]
    wd1t = Wd1.transpose(0, 3, 1, 2).reshape(P, FP + FN, C * FP).copy()
    wd2t = Wd2.transpose(0, 1, 3, 2).copy()                   # [P, C, FP, FP]
    bd1c = bd1.reshape(P, C, FP, 1).copy()
    bd2c = bd2.reshape(P, C, FP, 1).copy()
    iota = np.tile(np.arange(128, dtype=np.float32), (128, 1)).copy()
    ident = np.eye(128, dtype=np.float32)

    meta = dict(cfg=cfg, M_LOC=M_LOC, N_LOC=N_LOC, NH=NH, NHP=NHP,
                NB=NB, K_UP=K_UP, NBK=NBK, NTAB=NTAB, TRASH=TRASH,
                NCALLS=NCALLS, L=L, S_MAX=S_MAX)

    shared = dict(x=x_flat, wn1t=wn1t, wn2t=wn2t, wentA=wentA, wentB=wentB,
                  bn1c=bn1c, bn2c=bn2c, we1=we1, wd1t=wd1t, wd2t=wd2t,
                  bd1c=bd1c, bd2c=bd2c, iota=iota, ident=ident)
    in_maps = []
    for k in range(NC):
        m = dict(shared)
        m.update(xloc=xloc[k], up_src=up_src[k], up_dr=up_dr[k],
                 dn_dst=dn_dst[k], dn_srel=dn_srel[k], dn_scat=dn_scat[k],
                 degw=degw[k])
        in_maps.append(m)
    return in_maps, meta


def build_kernel(meta):
    cfg = meta["cfg"]
    P, N, M, E, C, FP, FN, NC = (cfg[k] for k in
                                 ("P", "N", "M", "E", "C", "FP", "FN", "NC"))
    M_LOC, NH, NHP = meta["M_LOC"], meta["NH"], meta["NHP"]
    NMT = NHP // 128
    NB, K_UP, NBK = meta["NB"], meta["K_UP"], meta["NBK"]
    NTAB, NCALLS, L = meta["NTAB"], meta["NCALLS"], meta["L"]
    CF = C * FP
    CN = C * FN
    NBW = FN + C * P           # nbt rows: class-4 n (FN) + b stack (C*P)
    assert C == 5

    nc = bacc.Bacc("TRN2", num_devices=NC)

    def param(name, shape, dt=F32, out=False):
        return nc.declare_dram_parameter(name, list(shape), dt, isOutput=out)

    x_d = param("x", [P * N, CF])
    xloc_d = param("xloc", [P, 2, CF, NH])
    up_src_d = param("up_src", [P, 128, NBK], I32)
    up_dr_d = param("up_dr", [P, 128, NBK])
    dn_dst_d = param("dn_dst", [2 * P, 128, L // 16], I16)
    dn_srel_d = param("dn_srel", [2 * P, 128, L // 16], I16)
    dn_scat_d = param("dn_scat", [2 * P, 128, L // 16], I16)
    degw_d = param("degw", [128, 2 * P * (NTAB // 128)])
    wn1t_d = param("wn1t", [P, FP, CN])
    wn2t_d = param("wn2t", [FN, CN])
    wentA_d = param("wentA", [4 * FN, C * P])
    wentB_d = param("wentB", [FN + 1, C * P])
    bn1c_d = param("bn1c", [C, FN, 1])
    bn2c_d = param("bn2c", [C, FN, 1])
    we1_d = param("we1", [P, FP, C])
    wd1t_d = param("wd1t", [P, FP + FN, C * FP])
    wd2t_d = param("wd2t", [P, C, FP, FP])
    bd1c_d = param("bd1c", [P, C, FP, 1])
    bd2c_d = param("bd2c", [P, C, FP, 1])
    iota_d = param("iota", [128, 128])
    ident_d = param("ident", [128, 128])
    out_d = param("outT", [P, 2, C, FP, NHP], out=True)

    n_loc = nc.dram_tensor("n_loc", [M_LOC, NROW], F32)
    n_full = nc.dram_tensor("n_full", [NC * M_LOC, NROW], F32,
                            addr_space="Shared")
    a_tabs = [nc.dram_tensor(f"a_tab{i}", [NTAB, AROW], F32)
              for i in range(2 * P)]
    s_tabs = [nc.dram_tensor(f"s_tab{i}", [NTAB, NROW], F32)
              for i in range(2 * P)]

    with tile.TileContext(nc) as tc:
        with tc.tile_pool(name="const", bufs=1) as cp:
            iota_t = cp.tile([128, 128], F32R)
            nc.sync.dma_start(out=iota_t[:], in_=iota_d[:].bitcast(F32R))
            ident_t = cp.tile([128, 128], F32)
            nc.sync.dma_start(out=ident_t[:], in_=ident_d[:])
            wn1t_t = [cp.tile([FP, CN], F32R, name=f"wn1t{p}")
                      for p in range(P)]
            wn2t_t = cp.tile([FN, CN], F32R)
            wentA_t = cp.tile([4 * FN, C * P], F32R)
            wentB_t = cp.tile([FN + 1, C * P], F32R)
            nc.sync.dma_start(out=wn2t_t[:], in_=wn2t_d[:].bitcast(F32R))
            nc.sync.dma_start(out=wentA_t[:], in_=wentA_d[:].bitcast(F32R))
            nc.sync.dma_start(out=wentB_t[:], in_=wentB_d[:].bitcast(F32R))
            bn1c_t = [cp.tile([FN, 1], F32, name=f"bn1c{c}") for c in range(C)]
            bn2c_t = [cp.tile([FN, 1], F32, name=f"bn2c{c}") for c in range(C)]
            we1_t = [cp.tile([FP, C], F32, name=f"we1{p}") for p in range(P)]
            wd1t_t = [cp.tile([FP + FN, C * FP], F32R, name=f"wd1t{p}")
                      for p in range(P)]
            wd2t_t = [[cp.tile([FP, FP], F32R, name=f"wd2t{p}_{c}")
                       for c in range(C)] for p in range(P)]
            bd1c_t = [[cp.tile([FP, 1], F32, name=f"bd1c{p}_{c}")
                       for c in range(C)] for p in range(P)]
            bd2c_t = [[cp.tile([FP, 1], F32, name=f"bd2c{p}_{c}")
                       for c in range(C)] for p in range(P)]
            for p in range(P):
                nc.sync.dma_start(out=wn1t_t[p][:], in_=wn1t_d[p].bitcast(F32R))
                nc.sync.dma_start(out=we1_t[p][:], in_=we1_d[p])
                nc.sync.dma_start(out=wd1t_t[p][:], in_=wd1t_d[p].bitcast(F32R))
                for c in range(C):
                    nc.sync.dma_start(out=wd2t_t[p][c][:],
                                      in_=wd2t_d[p, c].bitcast(F32R))
                    nc.sync.dma_start(out=bd1c_t[p][c][:], in_=bd1c_d[p, c])
                    nc.sync.dma_start(out=bd2c_t[p][c][:], in_=bd2c_d[p, c])
            for c in range(C):
                nc.sync.dma_start(out=bn1c_t[c][:], in_=bn1c_d[c])
                nc.sync.dma_start(out=bn2c_t[c][:], in_=bn2c_d[c])
            upsrc_t = [cp.tile([128, NBK], I32, name=f"upsrc{p}")
                       for p in range(P)]
            updr_t = [cp.tile([128, NBK], F32, name=f"updr{p}")
                      for p in range(P)]
            for p in range(P):
                nc.scalar.dma_start(out=upsrc_t[p][:], in_=up_src_d[p])
                nc.scalar.dma_start(out=updr_t[p][:], in_=up_dr_d[p])
            degw_t = cp.tile([128, 2 * P * (NTAB // 128)], F32)
            nc.scalar.dma_start(out=degw_t[:], in_=degw_d[:])

            # zero-init s tables
            zt = cp.tile([128, NROW], F32)
            nc.vector.memset(zt[:], 0.0)
            ones_f = cp.tile([1, GRP * 128], F32)
            nc.vector.memset(ones_f[:], 1.0)
            ones_r = cp.tile([1, GRP * 128], F32R)
            nc.vector.tensor_copy(out=ones_r[:], in_=ones_f[:])
            zeros_r = cp.tile([128, 64], F32R)
            nc.vector.tensor_copy(out=zeros_r[:], in_=zt[:, :64])
            for i in range(2 * P):
                st3 = s_tabs[i].ap().rearrange("(t q) r -> t q r", q=128)
                for t in range(NTAB // 128):
                    nc.sync.dma_start(out=st3[t], in_=zt[:])

            # ======================= UP PHASE =======================
            n_loc_ap = n_loc.ap()
            with tc.tile_pool(name="up_sb", bufs=3) as up, \
                 tc.tile_pool(name="up_sb1", bufs=2) as up1, \
                 tc.tile_pool(name="up_ps", bufs=2, space="PSUM") as upp, \
                 tc.tile_pool(name="up_ps1", bufs=1, space="PSUM") as upp1, \
                 tc.tile_pool(name="mlp_ps", bufs=1, space="PSUM") as mpp:
                for g0 in range(0, NB, GRP):
                    gb = list(range(g0, min(g0 + GRP, NB)))
                    GW = len(gb) * 128
                    # per-plane per-class feature-major up tiles [64, GRP*128]
                    upX = [[up1.tile([FP, GRP * 128], F32R,
                                     name=f"upX{p}_{c}", tag=f"upX{p}_{c}")
                            for c in range(C)] for p in range(P)]
                    for p in range(P):
                        for bi, b in enumerate(gb):
                            pu = upp.tile([128, CF], F32, tag="pu",
                                          space="PSUM")
                            for kk in range(K_UP):
                                col = b * K_UP + kk
                                G = up.tile([128, CF], F32R, tag="G")
                                nc.gpsimd.indirect_dma_start(
                                    out=G[:], out_offset=None,
                                    in_=x_d[:].bitcast(F32R),
                                    in_offset=bass.IndirectOffsetOnAxis(
                                        ap=upsrc_t[p][:, col:col + 1], axis=0))
                                O = up.tile([128, 128], F32R, tag="O")
                                nc.vector.tensor_tensor(
                                    out=O[:],
                                    in0=updr_t[p][:, col:col + 1]
                                        .bitcast(F32R).to_broadcast([128, 128]),
                                    in1=iota_t[:],
                                    op=ALU.is_equal)
                                nc.tensor.matmul(out=pu[:], lhsT=O[:],
                                                 rhs=G[:], start=(kk == 0),
                                                 stop=(kk == K_UP - 1))
                            stg = up.tile([128, CF], F32, tag="stg")
                            nc.scalar.copy(out=stg[:], in_=pu[:])
                            csl = slice(bi * 128, (bi + 1) * 128)
                            for ti in range(3):
                                w = min(128, CF - ti * 128)
                                pt = upp1.tile([128, 128], F32, tag="ptr",
                                               space="PSUM")
                                nc.tensor.transpose(
                                    out=pt[:w, :],
                                    in_=stg[:, ti * 128:ti * 128 + w],
                                    identity=ident_t[:])
                                nc.vector.tensor_copy(
                                    out=upX[p][2 * ti][:, csl],
                                    in_=pt[0:FP, :])
                                if 2 * ti + 1 < C:
                                    nc.vector.tensor_copy(
                                        out=upX[p][2 * ti + 1][:, csl],
                                        in_=pt[FP:2 * FP, :])
                    # ---- nexus MLP over this group ----
                    n1c = [up.tile([FN, GRP * 128], F32R, name=f"n1c{c}",
                                   tag=f"n1c{c}") for c in range(C)]
                    for c in range(C):
                        pn1 = mpp.tile([FN, GRP * 128], F32, tag="pn1",
                                       space="PSUM", bufs=2)
                        for p in range(P):
                            nc.tensor.matmul(
                                out=pn1[:, :GW],
                                lhsT=wn1t_t[p][:, c * FN:(c + 1) * FN],
                                rhs=upX[p][c][:, :GW],
                                start=(p == 0), stop=(p == P - 1))
                        nc.scalar.activation(n1c[c][:, :GW], pn1[:, :GW],
                                             TANH, bias=bn1c_t[c][:])
                    n2s = up.tile([4 * FN, GRP * 128], F32R, tag="n2s")
                    nbt = up.tile([FN + 1, GRP * 128], F32R, tag="nbt")
                    nc.vector.tensor_copy(out=nbt[FN:FN + 1, :],
                                          in_=ones_r[:])
                    for c in range(C):
                        pn2 = mpp.tile([FN, GRP * 128], F32, tag="pn2",
                                       space="PSUM", bufs=2)
                        nc.tensor.matmul(
                            out=pn2[:, :GW],
                            lhsT=wn2t_t[:, c * FN:(c + 1) * FN],
                            rhs=n1c[c][:, :GW], start=True, stop=True)
                        dst = (n2s[c * FN:(c + 1) * FN, :GW] if c < 4
                               else nbt[0:FN, :GW])
                        nc.scalar.activation(dst, pn2[:, :GW],
                                             TANH, bias=bn2c_t[c][:])
                    pbv = mpp.tile([C * P, GRP * 128], F32, tag="misc",
                                   space="PSUM", bufs=1)
                    nc.tensor.matmul(out=pbv[:, :GW], lhsT=wentA_t[:],
                                     rhs=n2s[:, :GW], start=True, stop=False)
                    nc.tensor.matmul(out=pbv[:, :GW], lhsT=wentB_t[:],
                                     rhs=nbt[:, :GW], start=False, stop=True)
                    bt = up.tile([C * P, GRP * 128], F32, tag="bt")
                    nc.vector.tensor_copy(out=bt[:, :GW], in_=pbv[:, :GW])
                    # assemble + store n rows per block
                    for bi, b in enumerate(gb):
                        rows = min(128, M_LOC - b * 128)
                        sl = slice(bi * 128, bi * 128 + 128)
                        tp = mpp.tile([128, 4 * FN + FN + C * P], F32,
                                      tag="misc", space="PSUM", bufs=1)
                        nc.tensor.transpose(
                            out=tp[:, 0:4 * FN],
                            in_=n2s[:, sl].bitcast(F32),
                            identity=ident_t[:])
                        nc.tensor.transpose(
                            out=tp[:, 4 * FN:CN],
                            in_=nbt[0:FN, sl].bitcast(F32),
                            identity=ident_t[:FN, :FN])
                        nc.tensor.transpose(
                            out=tp[:, CN:CN + C * P],
                            in_=bt[:, sl],
                            identity=ident_t[:C * P, :C * P])
                        nrow = up.tile([128, NROW], F32, tag="nrow")
                        nc.vector.tensor_copy(out=nrow[:, 0:CN + C * P],
                                              in_=tp[:])
                        nc.vector.memset(nrow[:, CN + C * P:], 0.0)
                        nc.sync.dma_start(
                            out=n_loc_ap[b * 128:b * 128 + rows, :],
                            in_=nrow[:rows, :])

            # ================= AllGather n =================
            nc.gpsimd.collective_compute(
                "AllGather", ALU.bypass,
                replica_groups=[list(range(NC))],
                ins=[n_loc.ap().opt()], outs=[n_full.ap().opt()])

            # ================= STAGE A: a tables =================
            with tc.tile_pool(name="sa_sb", bufs=2) as sa, \
                 tc.tile_pool(name="sa_ps", bufs=2, space="PSUM") as sap:
                for ph in range(2 * P):
                    p, h = ph // 2, ph % 2
                    for ch0 in range(0, NHP, CHW):
                        cw = min(CHW, NHP - ch0)
                        rw = min(max(NH - ch0, 0), cw)   # real cols
                        xtc = [sa.tile([FP, CHW], F32, name=f"xtc{c}",
                                       tag=f"xtc{c}") for c in range(C)]
                        for c in range(C):
                            if rw < cw:
                                nc.vector.memset(xtc[c][:, rw:cw], 0.0)
                            if rw > 0:
                                nc.sync.dma_start(
                                    out=xtc[c][:, :rw],
                                    in_=xloc_d[p, h, c * FP:(c + 1) * FP,
                                               ch0:ch0 + rw])
                        for j in range(cw // 128):
                            t = ch0 // 128 + j
                            pa = sap.tile([128, C], F32, tag="pa",
                                          space="PSUM")
                            for c in range(C):
                                nc.tensor.matmul(
                                    out=pa[:, c:c + 1],
                                    lhsT=xtc[c][:, j * 128:(j + 1) * 128],
                                    rhs=we1_t[p][:, c:c + 1],
                                    start=True, stop=True)
                            ast = sa.tile([128, AROW], F32, tag="ast")
                            nc.vector.memset(ast[:, C + 1:], 0.0)
                            nc.vector.tensor_copy(out=ast[:, 0:C], in_=pa[:])
                            nc.vector.reciprocal(
                                out=ast[:, C:C + 1],
                                in_=degw_t[:, ph * (NTAB // 128) + t:
                                           ph * (NTAB // 128) + t + 1])
                            nc.sync.dma_start(
                                out=a_tabs[ph].ap()[t * 128:(t + 1) * 128, :],
                                in_=ast[:])
                    for t in range(NMT, NTAB // 128):
                        nc.sync.dma_start(
                            out=a_tabs[ph].ap()[t * 128:(t + 1) * 128, :],
                            in_=zt[:, :AROW])

            # ================= STAGE B: edge stream =================
            NSL = B_SC // 128
            W16 = B_SC // 16
            with tc.tile_pool(name="sb_idx", bufs=1) as ip, \
                 tc.tile_pool(name="sb_sb", bufs=6) as sbp:
                dst_t, srel_t, scat_t = [], [], []
                for ph in range(2 * P):
                    d = ip.tile([128, L // 16], I16, name=f"dt{ph}")
                    nc.scalar.dma_start(out=d[:], in_=dn_dst_d[ph])
                    s = ip.tile([128, L // 16], I16, name=f"srt{ph}")
                    nc.scalar.dma_start(out=s[:], in_=dn_srel_d[ph])
                    sc = ip.tile([128, L // 16], I16, name=f"sct{ph}")
                    nc.scalar.dma_start(out=sc[:], in_=dn_scat_d[ph])
                    dst_t.append(d)
                    srel_t.append(s)
                    scat_t.append(sc)
                for cix in range(NCALLS):
                    for ph in range(2 * P):
                        p = ph // 2
                        isl = slice(cix * W16, (cix + 1) * W16)
                        gn = sbp.tile([128, NSL, NROW], F32, tag="gn")
                        nc.gpsimd.dma_gather(
                            out_ap=gn[:], in_ap=n_full.ap()[:],
                            idxs_ap=dst_t[ph][:, isl],
                            num_idxs=B_SC, num_idxs_reg=B_SC, elem_size=NROW)
                        ga = sbp.tile([128, NSL, AROW], F32, tag="ga")
                        nc.gpsimd.dma_gather(
                            out_ap=ga[:], in_ap=a_tabs[ph].ap()[:],
                            idxs_ap=srel_t[ph][:, isl],
                            num_idxs=B_SC, num_idxs_reg=B_SC, elem_size=AROW)
                        lg = sbp.tile([128, NSL, C], F32, tag="lg")
                        nc.vector.tensor_tensor(
                            out=lg[:], in0=ga[:, :, 0:C],
                            in1=gn[:, :, CN + p:CN + p + (C - 1) * P + 1:P],
                            op=ALU.add)
                        mx = sbp.tile([128, NSL], F32, tag="mx")
                        nc.vector.tensor_reduce(out=mx[:], in_=lg[:],
                                                axis=mybir.AxisListType.X,
                                                op=ALU.max)
                        nc.vector.tensor_tensor(
                            out=lg[:], in0=lg[:],
                            in1=mx[:].to_broadcast([128, NSL, C]),
                            op=ALU.subtract)
                        ex = sbp.tile([128, NSL, C], F32, tag="ex")
                        nc.scalar.activation(ex[:], lg[:], EXP)
                        sm = sbp.tile([128, NSL], F32, tag="sm")
                        nc.vector.tensor_reduce(out=sm[:], in_=ex[:],
                                                axis=mybir.AxisListType.X,
                                                op=ALU.add)
                        nc.vector.reciprocal(out=sm[:], in_=sm[:])
                        nc.vector.tensor_tensor(out=sm[:], in0=sm[:],
                                                in1=ga[:, :, C],
                                                op=ALU.mult)
                        nc.vector.tensor_tensor(
                            out=ex[:], in0=ex[:],
                            in1=sm[:].to_broadcast([128, NSL, C]),
                            op=ALU.mult)
                        msg = sbp.tile([128, NSL, NROW], F32, tag="msg")
                        nc.vector.memset(msg[:, :, CN:], 0.0)
                        nc.vector.tensor_tensor(
                            out=msg[:, :, 0:CN].rearrange(
                                "a b (c f) -> a b c f", f=FN),
                            in0=gn[:, :, 0:CN].rearrange(
                                "a b (c f) -> a b c f", f=FN),
                            in1=ex[:].to_broadcast([128, NSL, C, FN]),
                            op=ALU.mult)
                        nc.gpsimd.dma_scatter_add(
                            out_ap=s_tabs[ph].ap()[:], in_ap=msg[:],
                            idxs_ap=scat_t[ph][:, isl],
                            num_idxs=B_SC, num_idxs_reg=B_SC, elem_size=NROW)

            # ================= STAGE C: down MLP =================
            with tc.tile_pool(name="sc_sb", bufs=3) as scb, \
                 tc.tile_pool(name="sc_ft", bufs=1) as ftp, \
                 tc.tile_pool(name="sc_ps", bufs=2, space="PSUM") as scp:
                for ph in range(2 * P):
                    p, h = ph // 2, ph % 2
                    ft = [ftp.tile([FP + FN, NHP], F32R, name=f"ft{c}",
                                   tag=f"ft{c}") for c in range(C)]
                    for c in range(C):
                        if NHP > NH:
                            nc.vector.tensor_copy(
                                out=ft[c][:, NH:],
                                in_=zeros_r[:FP + FN, :NHP - NH])
                        nc.sync.dma_start(
                            out=ft[c][0:FP, :NH],
                            in_=xloc_d[p, h, c * FP:(c + 1) * FP, :]
                                .bitcast(F32R))
                    for t in range(NMT):
                        st = scb.tile([128, NROW], F32, tag="st")
                        nc.sync.dma_start(
                            out=st[:],
                            in_=s_tabs[ph].ap()[t * 128:(t + 1) * 128, :])
                        t1 = scp.tile([128, 128], F32, tag="st1", space="PSUM")
                        nc.tensor.transpose(out=t1[:, 0:4 * FN],
                                            in_=st[:, 0:4 * FN],
                                            identity=ident_t[:])
                        for c in range(4):
                            nc.vector.tensor_copy(
                                out=ft[c][FP:FP + FN, t * 128:(t + 1) * 128],
                                in_=t1[c * FN:(c + 1) * FN, :])
                        t2 = scp.tile([FN, 128], F32, tag="st2", space="PSUM")
                        nc.tensor.transpose(out=t2[:],
                                            in_=st[:, 4 * FN:CN],
                                            identity=ident_t[:])
                        nc.vector.tensor_copy(
                            out=ft[4][FP:FP + FN, t * 128:(t + 1) * 128],
                            in_=t2[:])
                    for ch0 in range(0, NHP, CHW):
                        cw = min(CHW, NHP - ch0)
                        csl = slice(ch0, ch0 + cw)
                        for c in range(C):
                            hps = scp.tile([FP, CHW], F32, tag="hps",
                                           space="PSUM")
                            nc.tensor.matmul(
                                out=hps[:, :cw],
                                lhsT=wd1t_t[p][:, c * FP:(c + 1) * FP],
                                rhs=ft[c][:, csl], start=True, stop=True)
                            ht = scb.tile([FP, CHW], F32R, tag="ht")
                            nc.scalar.activation(ht[:, :cw], hps[:, :cw],
                                                 TANH, bias=bd1c_t[p][c][:])
                            ops_ = scp.tile([FP, CHW], F32, tag="ops",
                                            space="PSUM")
                            nc.tensor.matmul(
                                out=ops_[:, :cw], lhsT=wd2t_t[p][c][:],
                                rhs=ht[:, :cw], start=True, stop=True)
                            ot = scb.tile([FP, CHW], F32, tag="ot")
                            nc.scalar.activation(ot[:, :cw], ops_[:, :cw],
                                                 TANH, bias=bd2c_t[p][c][:])
                            nc.sync.dma_start(
                                out=out_d[p, h, c, :, csl],
                                in_=ot[:, :cw])

    nc.compile()
    return nc


_CACHE = {}


def _get_compiled(inputs, cfg):
    in_maps, meta = host_prep(inputs, cfg)
    key = (meta["K_UP"], meta["NCALLS"], meta["S_MAX"],
           tuple(sorted(cfg.items())))
    if key not in _CACHE:
        _CACHE[key] = build_kernel(meta)
    return _CACHE[key], in_maps, meta


def assemble_output(results, meta):
    cfg = meta["cfg"]
    P, N, C, FP, NC = (cfg[k] for k in ("P", "N", "C", "FP", "NC"))
    NH = meta["NH"]
    # results[k]["outT"]: [P, 2, C, FP, NHP]
    arr = np.stack([np.asarray(results[k]["outT"])[:, :, :, :, :NH]
                    for k in range(NC)])
    # [NC, P, 2, C, FP, NH] -> [P, NC, 2, NH, C, FP]
    out = arr.transpose(1, 0, 2, 5, 3, 4).reshape(P, N, C, FP)
    return np.ascontiguousarray(out)


def kernel(**inputs):
    from concourse.bass_utils import run_bass_kernel_spmd
    cfg = CFG_FULL
    nc, in_maps, meta = _get_compiled(inputs, cfg)
    res = run_bass_kernel_spmd(nc, in_maps, list(range(cfg["NC"])))
    return assemble_output(res.results, meta)



# revision 4
# speedup vs baseline: 1.4763x; 1.1225x over previous
"""NexusNet GNN message-passing kernel for 8 Trainium2 NeuronCores. v2.

Sharding:
  - nexus_up + nexus MLP: sharded by nexus node (M/8 contiguous segs/core);
    edges routed to the core owning their dst segment (host index prep).
    Aggregation via one-hot matmul on PE into PSUM per 128-seg block.
  - n [M,C,FN] (+ per-plane edge-logit b terms, plane-major) AllGathered.
  - down phase (per plane/half, merged A+B+C): edges CSR-sorted by src and
    chunk-aligned per 128-node src block (chunk count per block = max over
    cores, so the program is SPMD-static).  Per 1024-edge call: dma_gather
    n[dst] rows; two one-hot matrices from srel (edge->src-in-block);
    a[src]+invdeg fetched per edge via one-hot matmul against a per-block
    a-table computed on PE from x; softmax weights on DVE; messages
    aggregated feature-major into per-block PSUM via msg^T @ onehot —
    no dma_scatter_add, no DRAM round-trip.
  - Final 2-layer MLP with class-pair packing ([128,512] tiles).
"""

import numpy as np

import concourse.bass as bass
import concourse.bacc as bacc
import concourse.mybir as mybir
import concourse.tile as tile

F32 = mybir.dt.float32
F32R = mybir.dt.float32r
BF16 = mybir.dt.bfloat16
I32 = mybir.dt.int32
I16 = mybir.dt.int16
TANH = mybir.ActivationFunctionType.Tanh
EXP = mybir.ActivationFunctionType.Exp
ALU = mybir.AluOpType

CFG_FULL = dict(P=3, N=100000, M=30000, E=200000, C=5, FP=64, FN=32, NC=8)

NROW = 256            # n-row bf16 elems (160 n + 15 b plane-major + pad)
GRP = 4               # up-phase seg blocks per nexus-MLP group
CHW = 512             # MLP chunk width


def _ceil(a, b):
    return (a + b - 1) // b


def _wrap16(a):
    # flat idx j -> (partition j%16, col j//16), replicated to 128 partitions
    w = a.reshape(-1, 16).T.copy()
    return np.tile(w, (8, 1))


def host_prep(inputs, cfg):
    P, N, M, E, C, FP, FN, NC = (cfg[k] for k in
                                 ("P", "N", "M", "E", "C", "FP", "FN", "NC"))
    M_LOC = M // NC
    N_LOC = N // NC
    NH = N_LOC // 2                       # nodes per half
    NHP = _ceil(NH, 128) * 128            # padded half (6272)
    NMT = NHP // 128                      # src blocks per half (49)
    NB = _ceil(M_LOC, 128)                # up seg blocks per core
    CF = C * FP

    x = np.ascontiguousarray(np.asarray(inputs["x"], np.float32)
                             .reshape(P, N, C * FP))
    esrc = np.asarray(inputs["edge_src"])
    edst = np.asarray(inputs["edge_dst"])

    import ml_dtypes
    # per-core feature-major x slices: [P, 2, C*FP, NH]  (bf16 for down phase)
    xloc = x.reshape(P, NC, 2, NH, C * FP).transpose(1, 0, 2, 4, 3)
    xloc_bf = np.ascontiguousarray(xloc).astype(ml_dtypes.bfloat16)
    CFP = _ceil(C * FP, 128) * 128        # x row padded to 256B multiple
    x_flat = np.zeros((P * N, CFP), ml_dtypes.bfloat16)
    x_flat[:, :C * FP] = x.reshape(P * N, C * FP).astype(ml_dtypes.bfloat16)

    # ------- UP phase indices: quarter-table dma_gather layout -------
    Q = 4
    QR = N // Q                           # x rows per (plane, quarter)
    cntq = np.zeros((NC, P, NB, Q), np.int64)
    per_kpq = {}
    for p in range(P):
        order = np.argsort(edst[p], kind="stable")
        ds, ss = edst[p][order], esrc[p][order]
        bounds = np.searchsorted(ds, np.arange(NC + 1) * M_LOC)
        for k in range(NC):
            sl = slice(bounds[k], bounds[k + 1])
            dsl = (ds[sl] - k * M_LOC).astype(np.int64)
            s = ss[sl].astype(np.int64)
            blk = dsl >> 7
            q = s // QR
            np.add.at(cntq[k, p], (blk, q), 1)
            per_kpq[(k, p)] = (blk, q, s % QR, dsl - (blk << 7))
    K_q = -(-cntq.max(axis=0) // 128)          # [P, NB, Q] chunks
    tot0 = K_q.sum(axis=2) == 0
    K_q[:, :, 0][tot0] = 1                     # every block gets >=1 chunk
    # group-align to 8-chunk (1024-idx) calls: every dma_gather is 1024 idxs
    NG = _ceil(NB, GRP)
    CW8 = np.zeros((P, Q, NG), np.int64)
    for p in range(P):
        for q in range(Q):
            for gi in range(NG):
                w = int(K_q[p, gi * GRP:(gi + 1) * GRP, q].sum())
                CW8[p, q, gi] = _ceil(w, 8) * 8
    gcol0 = np.zeros((P, Q, NG + 1), np.int64)
    gcol0[:, :, 1:] = np.cumsum(CW8, axis=2)
    bcol0 = np.zeros((P, Q, NB + 1), np.int64)
    for p in range(P):
        for q in range(Q):
            for b in range(NB):
                gi = b // GRP
                bcol0[p, q, b] = (gcol0[p, q, gi] +
                                  K_q[p, gi * GRP:b, q].sum())
            bcol0[p, q, NB] = gcol0[p, q, NG]
    NBQ = gcol0[:, :, NG]                      # [P, Q] padded cols per (p,q)
    NBQmax = int(NBQ.max())
    upq16 = np.zeros((NC, P, Q, 128, NBQmax * 8), np.int16)
    upqdr = np.full((NC, P, Q, 128, NBQmax), -1.0, np.float32)
    for (k, p), (blk, q, srow, dr) in per_kpq.items():
        key = blk * Q + q
        o2 = np.argsort(key, kind="stable")
        ks = key[o2]
        kcnt = np.bincount(ks, minlength=NB * Q)
        starts = np.concatenate(([0], np.cumsum(kcnt)))[:-1]
        rank = np.arange(len(ks)) - np.repeat(starts[np.unique(ks)],
                                              kcnt[np.unique(ks)])
        for qq in range(Q):
            nw = int(NBQ[p, qq])
            srcf = np.zeros(nw * 128, np.int64)
            drf = np.full(nw * 128, -1.0, np.float32)
            m = q[o2] == qq
            jj = bcol0[p, qq, blk[o2][m]] * 128 + rank[m]
            srcf[jj] = srow[o2][m]
            drf[jj] = dr[o2][m]
            upq16[k, p, qq, :, :nw * 8] = _wrap16(srcf.astype(np.int16))
            upqdr[k, p, qq, :, :nw] = drf.reshape(nw, 128).T
    upq16 = upq16.reshape(NC, P * Q, 128, NBQmax * 8) \
        .transpose(0, 2, 1, 3).reshape(NC, 128, -1).copy()
    upqdr = upqdr.reshape(NC, P * Q, 128, NBQmax) \
        .transpose(0, 2, 1, 3).reshape(NC, 128, -1).copy()

    # ---------------- DOWN phase indices (v2) ----------------
    down = {}
    cnt_all = np.zeros((NC, 2 * P, NMT), np.int64)
    for p in range(P):
        order = np.argsort(esrc[p], kind="stable")
        ss, dd = esrc[p][order], edst[p][order]
        bounds = np.searchsorted(ss, np.arange(2 * NC + 1) * NH)
        for j in range(2 * NC):
            k, h = j // 2, j % 2
            sl = slice(bounds[j], bounds[j + 1])
            s_loc = (ss[sl] - j * NH).astype(np.int64)
            d_loc = dd[sl].astype(np.int64)
            blk = s_loc >> 7
            cnt_all[k, p * 2 + h] = np.bincount(blk, minlength=NMT)
            down[(k, p * 2 + h)] = (s_loc, d_loc)
    # static chunk structure: per (ph, t) chunk count = max over cores
    K_t = np.maximum(1, -(-cnt_all.max(axis=0) // 128))   # [2P, NMT]
    NCH = K_t.sum(axis=1)                     # chunks per ph
    NCALLD = int(_ceil(int(NCH.max()), 8))    # calls per ph (common)
    NCHP = NCALLD * 8
    L = NCHP * 128
    # chunk -> block map (per ph), pads assigned to last block
    chunks = np.full((2 * P, NCHP), NMT - 1, np.int64)
    cstart = np.zeros((2 * P, NMT), np.int64)
    for ph in range(2 * P):
        pos = 0
        for t in range(NMT):
            cstart[ph, t] = pos
            chunks[ph, pos:pos + K_t[ph, t]] = t
            pos += K_t[ph, t]
    # first/last chunk flags per (ph, chunk)
    first = np.zeros((2 * P, NCHP), bool)
    last = np.zeros((2 * P, NCHP), bool)
    for ph in range(2 * P):
        for t in range(NMT):
            first[ph, cstart[ph, t]] = True
            e = cstart[ph, t + 1] if t + 1 < NMT else NCHP
            last[ph, e - 1] = True

    dn16 = np.zeros((NC, 2 * P, 128, L // 16), np.int16)
    srel_col = np.full((NC, 2 * P, 128, NCHP), 999.0, np.float32)
    srep = np.full((NC, 2 * P, NCALLD, 1024), 999.0, np.float32)
    invdeg = np.ones((NC, 2 * P, 128, NMT), np.float32)
    for (k, ph), (s_loc, d_loc) in down.items():
        dstA = np.zeros(L, np.int64)
        srelA = np.full(L, 999.0, np.float32)
        blk = s_loc >> 7
        cnt = np.bincount(blk, minlength=NMT)
        starts = np.concatenate(([0], np.cumsum(cnt)))[:-1]
        r = np.arange(len(s_loc)) - np.repeat(starts, cnt)
        pos = np.repeat(cstart[ph] * 128, cnt) + r
        dstA[pos] = d_loc
        srelA[pos] = (s_loc - (blk << 7)).astype(np.float32)
        dn16[k, ph] = _wrap16(dstA.astype(np.int16))
        srel_col[k, ph] = srelA.reshape(NCHP, 128).T
        srep[k, ph] = srelA.reshape(NCALLD, 1024)
        deg = np.bincount(s_loc, minlength=NHP)
        invdeg[k, ph] = (1.0 / np.maximum(deg, 1)).astype(
            np.float32).reshape(NMT, 128).T
    # flatten idx tensors for single-tile loads
    dn16f = dn16.reshape(NC, 2 * P, 128, L // 16).transpose(0, 2, 1, 3) \
        .reshape(NC, 128, -1).copy()
    srcolf = srel_col.transpose(0, 2, 1, 3).reshape(NC, 128, -1).copy()
    invdegf = invdeg.transpose(0, 2, 1, 3).reshape(NC, 128, -1).copy()

    # ---------------- weights ----------------
    g = lambda n: np.asarray(inputs[n], np.float32)
    Wn1, Wn2, We, Wd1, Wd2 = g("Wn1"), g("Wn2"), g("We"), g("Wd1"), g("Wd2")
    bn1, bn2, be, bd1, bd2 = g("bn1"), g("bn2"), g("be"), g("bd1"), g("bd2")

    wn1t = np.stack([Wn1.transpose(2, 0, 1)[p * FP:(p + 1) * FP]
                     .reshape(FP, C * FN) for p in range(P)]).copy()
    wn2t = Wn2.transpose(2, 0, 1).reshape(FN, C * FN).copy()
    # b-term weights, PLANE-major output cols (col = p*C + c)
    went = We[:, :, 0, FP:]                                   # [P, C, FN]
    wentA = np.zeros((4 * FN, C * P), np.float32)
    for c in range(4):
        for p in range(P):
            wentA[c * FN:(c + 1) * FN, p * C + c] = went[p, c, :]
    wentB = np.zeros((FN + 1, C * P), np.float32)
    for p in range(P):
        wentB[:FN, p * C + 4] = went[p, 4, :]
        wentB[FN, p * C:(p + 1) * C] = be[p, :, 0]
    bn1c = bn1.reshape(C, FN, 1).copy()
    bn2c = bn2.reshape(C, FN, 1).copy()

    bf = ml_dtypes.bfloat16
    we1 = We[:, :, 0, :FP]                                    # [P, C, FP]
    we1pair = np.zeros((P, 2, 128, 2), np.float32)
    for p in range(P):
        for gidx in range(2):
            for a in range(2):
                we1pair[p, gidx, a * FP:(a + 1) * FP, a] = \
                    we1[p, 2 * gidx + a]
    we1_4 = we1[:, 4, :].reshape(P, FP, 1).copy()
    wd1x = np.zeros((P, 2, 128, 128), np.float32)
    wd1s = np.zeros((P, 2, 2 * FN, 128), np.float32)
    wd2p = np.zeros((P, 2, 128, 128), np.float32)
    bd1p = np.zeros((P, 2, 128, 1), np.float32)
    bd2p = np.zeros((P, 2, 128, 1), np.float32)
    for p in range(P):
        for gi in range(2):
            for a in range(2):
                c = 2 * gi + a
                wd1x[p, gi, a * FP:(a + 1) * FP, a * FP:(a + 1) * FP] = \
                    Wd1[p, c, :, 0:FP].T
                wd1s[p, gi, a * FN:(a + 1) * FN, a * FP:(a + 1) * FP] = \
                    Wd1[p, c, :, FP:].T
                wd2p[p, gi, a * FP:(a + 1) * FP, a * FP:(a + 1) * FP] = \
                    Wd2[p, c].T
                bd1p[p, gi, a * FP:(a + 1) * FP, 0] = bd1[p, c]
                bd2p[p, gi, a * FP:(a + 1) * FP, 0] = bd2[p, c]
    wd1x4 = Wd1[:, 4, :, 0:FP].transpose(0, 2, 1).copy()      # [P, FP, FP]
    wd1s4 = Wd1[:, 4, :, FP:].transpose(0, 2, 1).copy()       # [P, FN, FP]
    wd2_4 = Wd2[:, 4].transpose(0, 2, 1).copy()               # [P, FP, FP]
    bd1_4 = bd1[:, 4].reshape(P, FP, 1).copy()
    bd2_4 = bd2[:, 4].reshape(P, FP, 1).copy()

    iota = np.tile(np.arange(128, dtype=np.float32), (128, 1)).copy()
    iotac = np.arange(128, dtype=np.float32).reshape(128, 1).copy()
    ident = np.eye(128, dtype=np.float32)

    tobf = lambda a: np.ascontiguousarray(a.astype(bf))

    meta = dict(cfg=cfg, M_LOC=M_LOC, N_LOC=N_LOC, NH=NH, NHP=NHP, NMT=NMT,
                NB=NB, CFP=CFP, NCALLD=NCALLD, NCHP=NCHP, L=L,
                chunks=chunks, first=first, last=last,
                K_q=K_q, NBQ=NBQ, NBQmax=NBQmax, bcol0=bcol0,
                gcol0=gcol0, CW8=CW8, Q=Q)

    shared = dict(x=x_flat, wn1t=tobf(wn1t), wn2t=tobf(wn2t),
                  wentA=wentA, wentB=wentB,
                  bn1c=bn1c, bn2c=bn2c, iota=iota, iotac=iotac, ident=ident,
                  we1pair=tobf(we1pair), we1_4=tobf(we1_4),
                  wd1x=tobf(wd1x), wd1s=tobf(wd1s), wd2p=tobf(wd2p),
                  wd1x4=tobf(wd1x4), wd1s4=tobf(wd1s4), wd2_4=tobf(wd2_4),
                  bd1p=bd1p, bd2p=bd2p, bd1_4=bd1_4, bd2_4=bd2_4)
    in_maps = []
    for k in range(NC):
        m = dict(shared)
        m.update(xloc=np.ascontiguousarray(xloc_bf[k]),
                 upq16=upq16[k], upqdr=upqdr[k],
                 dn16=dn16f[k], srcol=tobf(srcolf[k]), srep=tobf(srep[k]),
                 invdeg=invdegf[k])
        in_maps.append(m)
    return in_maps, meta


def build_kernel(meta):
    cfg = meta["cfg"]
    P, N, M, E, C, FP, FN, NC = (cfg[k] for k in
                                 ("P", "N", "M", "E", "C", "FP", "FN", "NC"))
    M_LOC, NH, NHP, NMT = meta["M_LOC"], meta["NH"], meta["NHP"], meta["NMT"]
    NB = meta["NB"]
    NCALLD, NCHP, L = meta["NCALLD"], meta["NCHP"], meta["L"]
    chunks, first, last = meta["chunks"], meta["first"], meta["last"]
    K_q, NBQ, NBQmax = meta["K_q"], meta["NBQ"], meta["NBQmax"]
    bcol0, Q = meta["bcol0"], meta["Q"]
    gcol0, CW8 = meta["gcol0"], meta["CW8"]
    CFP = meta["CFP"]
    QR = N // Q
    CF = C * FP
    CN = C * FN
    assert C == 5

    nc = bacc.Bacc("TRN2", num_devices=NC)

    def param(name, shape, dt=F32, out=False):
        return nc.declare_dram_parameter(name, list(shape), dt, isOutput=out)

    x_d = param("x", [P * N, CFP], BF16)
    xloc_d = param("xloc", [P, 2, CF, NH], BF16)
    upq16_d = param("upq16", [128, P * Q * NBQmax * 8], I16)
    upqdr_d = param("upqdr", [128, P * Q * NBQmax])
    dn16_d = param("dn16", [128, 2 * P * (L // 16)], I16)
    srcol_d = param("srcol", [128, 2 * P * NCHP], BF16)
    srep_d = param("srep", [2 * P, NCALLD, 1024], BF16)
    invdeg_d = param("invdeg", [128, 2 * P * NMT])
    wn1t_d = param("wn1t", [P, FP, CN], BF16)
    wn2t_d = param("wn2t", [FN, CN], BF16)
    wentA_d = param("wentA", [4 * FN, C * P])
    wentB_d = param("wentB", [FN + 1, C * P])
    bn1c_d = param("bn1c", [C, FN, 1])
    bn2c_d = param("bn2c", [C, FN, 1])
    we1pair_d = param("we1pair", [P, 2, 128, 2], BF16)
    we1_4_d = param("we1_4", [P, FP, 1], BF16)
    wd1x_d = param("wd1x", [P, 2, 128, 128], BF16)
    wd1s_d = param("wd1s", [P, 2, 2 * FN, 128], BF16)
    wd2p_d = param("wd2p", [P, 2, 128, 128], BF16)
    wd1x4_d = param("wd1x4", [P, FP, FP], BF16)
    wd1s4_d = param("wd1s4", [P, FN, FP], BF16)
    wd2_4_d = param("wd2_4", [P, FP, FP], BF16)
    bd1p_d = param("bd1p", [P, 2, 128, 1])
    bd2p_d = param("bd2p", [P, 2, 128, 1])
    bd1_4_d = param("bd1_4", [P, FP, 1])
    bd2_4_d = param("bd2_4", [P, FP, 1])
    iota_d = param("iota", [128, 128])
    iotac_d = param("iotac", [128, 1])
    ident_d = param("ident", [128, 128])
    out_d = param("outT", [P, 2, C, FP, NHP], out=True)

    n_loc = nc.dram_tensor("n_loc", [M_LOC, NROW], BF16)
    n_full = nc.dram_tensor("n_full", [NC * M_LOC, NROW], BF16,
                            addr_space="Shared")

    with tile.TileContext(nc) as tc:
        with tc.tile_pool(name="const", bufs=1) as cp:
            iota_t = cp.tile([128, 128], F32R)
            nc.sync.dma_start(out=iota_t[:], in_=iota_d[:].bitcast(F32R))
            iotac_t = cp.tile([128, 1], F32R)
            nc.sync.dma_start(out=iotac_t[:], in_=iotac_d[:].bitcast(F32R))
            ident_t = cp.tile([128, 128], F32)
            nc.sync.dma_start(out=ident_t[:], in_=ident_d[:])
            wn1t_t = [cp.tile([FP, CN], BF16, name=f"wn1t{p}")
                      for p in range(P)]
            wn2t_t = cp.tile([FN, CN], BF16)
            wentA_t = cp.tile([4 * FN, C * P], F32R)
            wentB_t = cp.tile([FN + 1, C * P], F32R)
            nc.sync.dma_start(out=wn2t_t[:], in_=wn2t_d[:])
            nc.sync.dma_start(out=wentA_t[:], in_=wentA_d[:].bitcast(F32R))
            nc.sync.dma_start(out=wentB_t[:], in_=wentB_d[:].bitcast(F32R))
            bn1c_t = [cp.tile([FN, 1], F32, name=f"bn1c{c}") for c in range(C)]
            bn2c_t = [cp.tile([FN, 1], F32, name=f"bn2c{c}") for c in range(C)]
            for c in range(C):
                nc.sync.dma_start(out=bn1c_t[c][:], in_=bn1c_d[c])
                nc.sync.dma_start(out=bn2c_t[c][:], in_=bn2c_d[c])
            for p in range(P):
                nc.sync.dma_start(out=wn1t_t[p][:], in_=wn1t_d[p])
            upq16_t = cp.tile([128, P * Q * NBQmax * 8], I16)
            nc.scalar.dma_start(out=upq16_t[:], in_=upq16_d[:])
            upqdr_t = cp.tile([128, P * Q * NBQmax], F32)
            nc.scalar.dma_start(out=upqdr_t[:], in_=upqdr_d[:])
            # down-phase constants
            we1pair_t = [[cp.tile([128, 2], BF16, name=f"we1p{p}_{gi}")
                          for gi in range(2)] for p in range(P)]
            we1_4_t = [cp.tile([FP, 1], BF16, name=f"we14{p}")
                       for p in range(P)]
            wd1x_t = [[cp.tile([128, 128], BF16, name=f"wd1x{p}_{gi}")
                       for gi in range(2)] for p in range(P)]
            wd1s_t = [[cp.tile([2 * FN, 128], BF16, name=f"wd1s{p}_{gi}")
                       for gi in range(2)] for p in range(P)]
            wd2p_t = [[cp.tile([128, 128], BF16, name=f"wd2p{p}_{gi}")
                       for gi in range(2)] for p in range(P)]
            wd1x4_t = [cp.tile([FP, FP], BF16, name=f"wd1x4{p}")
                       for p in range(P)]
            wd1s4_t = [cp.tile([FN, FP], BF16, name=f"wd1s4{p}")
                       for p in range(P)]
            wd2_4_t = [cp.tile([FP, FP], BF16, name=f"wd24{p}")
                       for p in range(P)]
            bd1p_t = [[cp.tile([128, 1], F32, name=f"bd1p{p}_{gi}")
                       for gi in range(2)] for p in range(P)]
            bd2p_t = [[cp.tile([128, 1], F32, name=f"bd2p{p}_{gi}")
                       for gi in range(2)] for p in range(P)]
            bd1_4_t = [cp.tile([FP, 1], F32, name=f"bd14{p}")
                       for p in range(P)]
            bd2_4_t = [cp.tile([FP, 1], F32, name=f"bd24{p}")
                       for p in range(P)]
            for p in range(P):
                for gi in range(2):
                    nc.sync.dma_start(out=we1pair_t[p][gi][:],
                                      in_=we1pair_d[p, gi])
                    nc.sync.dma_start(out=wd1x_t[p][gi][:], in_=wd1x_d[p, gi])
                    nc.sync.dma_start(out=wd1s_t[p][gi][:], in_=wd1s_d[p, gi])
                    nc.sync.dma_start(out=wd2p_t[p][gi][:], in_=wd2p_d[p, gi])
                    nc.sync.dma_start(out=bd1p_t[p][gi][:], in_=bd1p_d[p, gi])
                    nc.sync.dma_start(out=bd2p_t[p][gi][:], in_=bd2p_d[p, gi])
                nc.sync.dma_start(out=we1_4_t[p][:], in_=we1_4_d[p])
                nc.sync.dma_start(out=wd1x4_t[p][:], in_=wd1x4_d[p])
                nc.sync.dma_start(out=wd1s4_t[p][:], in_=wd1s4_d[p])
                nc.sync.dma_start(out=wd2_4_t[p][:], in_=wd2_4_d[p])
                nc.sync.dma_start(out=bd1_4_t[p][:], in_=bd1_4_d[p])
                nc.sync.dma_start(out=bd2_4_t[p][:], in_=bd2_4_d[p])
            dn16_t = cp.tile([128, 2 * P * (L // 16)], I16)
            nc.scalar.dma_start(out=dn16_t[:], in_=dn16_d[:])
            srcol_t = cp.tile([128, 2 * P * NCHP], BF16)
            nc.scalar.dma_start(out=srcol_t[:], in_=srcol_d[:])
            iotab_t = cp.tile([128, 128], BF16)
            nc.vector.tensor_copy(out=iotab_t[:], in_=iota_t[:].bitcast(F32))
            iotacb_t = cp.tile([128, 1], BF16)
            nc.vector.tensor_copy(out=iotacb_t[:],
                                  in_=iotac_t[:].bitcast(F32))
            invdeg_t = cp.tile([128, 2 * P * NMT], F32)
            nc.scalar.dma_start(out=invdeg_t[:], in_=invdeg_d[:])

            # ======================= UP PHASE =======================
            n_loc_ap = n_loc.ap()
            ones_f = cp.tile([1, GRP * 128], F32)
            nc.vector.memset(ones_f[:], 1.0)
            ones_r = cp.tile([1, GRP * 128], F32R)
            nc.vector.tensor_copy(out=ones_r[:], in_=ones_f[:])
            with tc.tile_pool(name="up_sb", bufs=2) as up, \
                 tc.tile_pool(name="up_sb1", bufs=2) as up1, \
                 tc.tile_pool(name="up_g", bufs=2) as upg, \
                 tc.tile_pool(name="up_ps", bufs=2, space="PSUM") as upp, \
                 tc.tile_pool(name="up_ps1", bufs=1, space="PSUM") as upp1, \
                 tc.tile_pool(name="mlp_ps", bufs=1, space="PSUM") as mpp:
                for g0 in range(0, NB, GRP):
                    gb = list(range(g0, min(g0 + GRP, NB)))
                    GW = len(gb) * 128
                    upX = [[up1.tile([FP, GRP * 128], BF16,
                                     name=f"upX{p}_{c}", tag=f"upX{p}_{c}")
                            for c in range(C)] for p in range(P)]
                    gidx = g0 // GRP
                    for p in range(P):
                        # constant 1024-idx dma_gather calls per quarter
                        Gq, Ob, qc0 = {}, {}, {}
                        for q in range(Q):
                            c0 = int(gcol0[p, q, gidx])
                            ncalls = int(CW8[p, q, gidx]) // 8
                            qc0[q] = c0
                            base = (p * Q + q) * NBQmax
                            Gq[q], Ob[q] = [], []
                            for jc in range(ncalls):
                                cj = c0 + 8 * jc
                                Gt = upg.tile([128, 8, CFP], BF16,
                                              tag=f"Gq{q}")
                                nc.gpsimd.dma_gather(
                                    out_ap=Gt[:],
                                    in_ap=x_d.ap()[(p * Q + q) * QR:
                                                   (p * Q + q + 1) * QR],
                                    idxs_ap=upq16_t[:, (base + cj) * 8:
                                                    (base + cj + 8) * 8],
                                    num_idxs=1024, num_idxs_reg=1024,
                                    elem_size=CFP)
                                Ot = upg.tile([128, 8, 128], BF16,
                                              tag=f"Ob{q}")
                                nc.vector.tensor_tensor(
                                    out=Ot[:],
                                    in0=upqdr_t[:, base + cj:base + cj + 8]
                                        .to_broadcast([128, 8, 128]),
                                    in1=iota_t[:].bitcast(F32)
                                        .rearrange("z (a e) -> z a e", a=1)
                                        .to_broadcast([128, 8, 128]),
                                    op=ALU.is_equal)
                                Gq[q].append(Gt)
                                Ob[q].append(Ot)
                        for bi, b in enumerate(gb):
                            pu = upp.tile([128, CF], F32, tag="pu",
                                          space="PSUM")
                            nchb = int(K_q[p, b, :].sum())
                            ci = 0
                            for q in range(Q):
                                for j in range(int(K_q[p, b, q])):
                                    col = int(bcol0[p, q, b]) + j - qc0[q]
                                    jc, sl = col // 8, col % 8
                                    nc.tensor.matmul(
                                        out=pu[:],
                                        lhsT=Ob[q][jc][:, sl, :],
                                        rhs=Gq[q][jc][:, sl, 0:CF],
                                        start=(ci == 0),
                                        stop=(ci == nchb - 1))
                                    ci += 1
                            stg = up.tile([128, CF], F32, tag="stg")
                            nc.scalar.copy(out=stg[:], in_=pu[:])
                            csl = slice(bi * 128, (bi + 1) * 128)
                            for ti in range(3):
                                w = min(128, CF - ti * 128)
                                pt = upp1.tile([128, 128], F32, tag="ptr",
                                               space="PSUM")
                                nc.tensor.transpose(
                                    out=pt[:w, :],
                                    in_=stg[:, ti * 128:ti * 128 + w],
                                    identity=ident_t[:])
                                nc.vector.tensor_copy(
                                    out=upX[p][2 * ti][:, csl],
                                    in_=pt[0:FP, :])
                                if 2 * ti + 1 < C:
                                    nc.vector.tensor_copy(
                                        out=upX[p][2 * ti + 1][:, csl],
                                        in_=pt[FP:2 * FP, :])
                    # ---- nexus MLP over this group ----
                    n1c = [up.tile([FN, GRP * 128], BF16, name=f"n1c{c}",
                                   tag=f"n1c{c}") for c in range(C)]
                    for c in range(C):
                        pn1 = mpp.tile([FN, GRP * 128], F32, tag="pn1",
                                       space="PSUM", bufs=2)
                        for p in range(P):
                            nc.tensor.matmul(
                                out=pn1[:, :GW],
                                lhsT=wn1t_t[p][:, c * FN:(c + 1) * FN],
                                rhs=upX[p][c][:, :GW],
                                start=(p == 0), stop=(p == P - 1))
                        nc.scalar.activation(n1c[c][:, :GW], pn1[:, :GW],
                                             TANH, bias=bn1c_t[c][:])
                    n2s = up.tile([4 * FN, GRP * 128], F32R, tag="n2s")
                    nbt = up.tile([FN + 1, GRP * 128], F32R, tag="nbt")
                    nc.vector.tensor_copy(out=nbt[FN:FN + 1, :],
                                          in_=ones_r[:])
                    for c in range(C):
                        pn2 = mpp.tile([FN, GRP * 128], F32, tag="pn2",
                                       space="PSUM", bufs=2)
                        nc.tensor.matmul(
                            out=pn2[:, :GW],
                            lhsT=wn2t_t[:, c * FN:(c + 1) * FN],
                            rhs=n1c[c][:, :GW], start=True, stop=True)
                        dst = (n2s[c * FN:(c + 1) * FN, :GW] if c < 4
                               else nbt[0:FN, :GW])
                        nc.scalar.activation(dst, pn2[:, :GW],
                                             TANH, bias=bn2c_t[c][:])
                    pbv = mpp.tile([C * P, GRP * 128], F32, tag="misc",
                                   space="PSUM", bufs=1)
                    nc.tensor.matmul(out=pbv[:, :GW], lhsT=wentA_t[:],
                                     rhs=n2s[:, :GW], start=True, stop=False)
                    nc.tensor.matmul(out=pbv[:, :GW], lhsT=wentB_t[:],
                                     rhs=nbt[:, :GW], start=False, stop=True)
                    bt = up.tile([C * P, GRP * 128], F32, tag="bt")
                    nc.vector.tensor_copy(out=bt[:, :GW], in_=pbv[:, :GW])
                    # assemble + store n rows per block
                    for bi, b in enumerate(gb):
                        rows = min(128, M_LOC - b * 128)
                        sl = slice(bi * 128, bi * 128 + 128)
                        tp = mpp.tile([128, 4 * FN + FN + C * P], F32,
                                      tag="misc", space="PSUM", bufs=1)
                        nc.tensor.transpose(
                            out=tp[:, 0:4 * FN],
                            in_=n2s[:, sl].bitcast(F32),
                            identity=ident_t[:])
                        nc.tensor.transpose(
                            out=tp[:, 4 * FN:CN],
                            in_=nbt[0:FN, sl].bitcast(F32),
                            identity=ident_t[:FN, :FN])
                        nc.tensor.transpose(
                            out=tp[:, CN:CN + C * P],
                            in_=bt[:, sl],
                            identity=ident_t[:C * P, :C * P])
                        nrow = up.tile([128, NROW], BF16, tag="nrow")
                        nc.vector.tensor_copy(out=nrow[:, 0:CN + C * P],
                                              in_=tp[:])
                        nc.vector.memset(nrow[:, CN + C * P:], 0.0)
                        nc.sync.dma_start(
                            out=n_loc_ap[b * 128:b * 128 + rows, :],
                            in_=nrow[:rows, :])

            # ================= DOWN PHASE =================
            with tc.tile_pool(name="dn_ft", bufs=2) as ftp, \
                 tc.tile_pool(name="dn_sf", bufs=1) as sfp, \
                 tc.tile_pool(name="dn_ab", bufs=2) as abp, \
                 tc.tile_pool(name="dn_sb", bufs=3) as sbp, \
                 tc.tile_pool(name="dn_o", bufs=2) as obp, \
                 tc.tile_pool(name="dn_ps", bufs=2, space="PSUM") as psp, \
                 tc.tile_pool(name="dn_sps", bufs=2, space="PSUM") as spp, \
                 tc.tile_pool(name="dn_mps", bufs=1, space="PSUM") as mpp2:

                def emit_prep(ph):
                    p, h = ph // 2, ph % 2
                    ftx01 = ftp.tile([128, NHP], BF16, tag="ftx01")
                    ftx23 = ftp.tile([128, NHP], BF16, tag="ftx23")
                    ftx4 = ftp.tile([FP, NHP], BF16, tag="ftx4")
                    for t_, lo in ((ftx01, 0), (ftx23, 128)):
                        nc.sync.dma_start(
                            out=t_[:, 0:NH],
                            in_=xloc_d[p, h, lo:lo + 128, :])
                        nc.vector.memset(t_[:, NH:NHP], 0.0)
                    nc.sync.dma_start(out=ftx4[:, 0:NH],
                                      in_=xloc_d[p, h, 256:320, :])
                    nc.vector.memset(ftx4[:, NH:NHP], 0.0)
                    # ---- per-block a-table [128, NMT*(C+1)] ----
                    ab_nm = abp.tile([128, NMT * (C + 1)], BF16, tag="ab")
                    for t in range(NMT):
                        pa = psp.tile([128, 8 * (C + 1)], F32, tag="aps",
                                      space="PSUM")
                        tsl = slice(t * 128, (t + 1) * 128)
                        nc.tensor.matmul(out=pa[:, 0:2],
                                         lhsT=ftx01[:, tsl],
                                         rhs=we1pair_t[p][0][:],
                                         start=True, stop=True)
                        nc.tensor.matmul(out=pa[:, 2:4],
                                         lhsT=ftx23[:, tsl],
                                         rhs=we1pair_t[p][1][:],
                                         start=True, stop=True)
                        nc.tensor.matmul(out=pa[:, 4:5],
                                         lhsT=ftx4[:, tsl],
                                         rhs=we1_4_t[p][:],
                                         start=True, stop=True)
                        nc.vector.tensor_copy(
                            out=ab_nm[:, t * (C + 1):t * (C + 1) + C],
                            in_=pa[:, 0:C])
                    nc.vector.tensor_copy(
                        out=ab_nm[:].rearrange("q (t u) -> q t u",
                                               u=C + 1)[:, :, C],
                        in_=invdeg_t[:, ph * NMT:(ph + 1) * NMT])
                    return ftx01, ftx23, ftx4, ab_nm

                prep0 = emit_prep(0)

                # ---- AllGather n (overlaps with ph0 prep above) ----
                nc.gpsimd.collective_compute(
                    "AllGather", ALU.bypass,
                    replica_groups=[list(range(NC))],
                    ins=[n_loc.ap().opt()], outs=[n_full.ap().opt()])

                for ph in range(2 * P):
                    p, h = ph // 2, ph % 2
                    ftx01, ftx23, ftx4, ab_nm = (prep0 if ph == 0
                                                 else emit_prep(ph))
                    sfm01 = sfp.tile([2 * FN, NHP], BF16, tag="sfm01")
                    sfm23 = sfp.tile([2 * FN, NHP], BF16, tag="sfm23")
                    sfm4 = sfp.tile([FN, NHP], BF16, tag="sfm4")
                    # ---- edge stream ----
                    sps1 = sps2 = None
                    for j in range(NCALLD):
                        gn = sbp.tile([128, 8, NROW], BF16, tag="gn")
                        ioff = ph * (L // 16) + j * 64
                        nc.gpsimd.dma_gather(
                            out_ap=gn[:], in_ap=n_full.ap()[:],
                            idxs_ap=dn16_t[:, ioff:ioff + 64],
                            num_idxs=1024, num_idxs_reg=1024, elem_size=NROW)
                        srt = sbp.tile([128, 1024], BF16, tag="srt")
                        nc.sync.dma_start(
                            out=srt[:],
                            in_=srep_d[ph, j:j + 1]
                            .to_broadcast([128, 1024]))
                        Ost = sbp.tile([128, 8, 128], BF16, tag="Ost")
                        coff = ph * NCHP + j * 8
                        nc.vector.tensor_tensor(
                            out=Ost[:],
                            in0=srcol_t[:, coff:coff + 8]
                                .to_broadcast([128, 8, 128]),
                            in1=iotab_t[:].rearrange("q (a e) -> q a e", a=1)
                                .to_broadcast([128, 8, 128]),
                            op=ALU.is_equal)
                        Ots = sbp.tile([128, 8, 128], BF16, tag="Ots")
                        nc.vector.tensor_tensor(
                            out=Ots[:],
                            in0=srt[:].rearrange("q (k e) -> q k e", k=8),
                            in1=iotacb_t[:].rearrange("q (a b) -> q a b", a=1)
                                .to_broadcast([128, 8, 128]),
                            op=ALU.is_equal)
                        a_ps = psp.tile([128, 8 * (C + 1)], F32, tag="aps",
                                        space="PSUM")
                        for k in range(8):
                            t = int(chunks[ph][j * 8 + k])
                            nc.tensor.matmul(
                                out=a_ps[:, k * (C + 1):(k + 1) * (C + 1)],
                                lhsT=Ots[:, k, :],
                                rhs=ab_nm[:, t * (C + 1):(t + 1) * (C + 1)],
                                start=True, stop=True)
                        apv = a_ps[:].rearrange("q (k u) -> q k u", u=C + 1)
                        lg = sbp.tile([128, 8, C], F32, tag="lg")
                        nc.vector.tensor_tensor(
                            out=lg[:], in0=apv[:, :, 0:C],
                            in1=gn[:, :, CN + p * C:CN + (p + 1) * C],
                            op=ALU.add)
                        ex = sbp.tile([128, 8, C], F32, tag="ex")
                        nc.scalar.activation(ex[:], lg[:], EXP)
                        sm = sbp.tile([128, 8], F32, tag="sm")
                        nc.vector.tensor_reduce(out=sm[:], in_=ex[:],
                                                axis=mybir.AxisListType.X,
                                                op=ALU.add)
                        nc.vector.reciprocal(out=sm[:], in_=sm[:])
                        nc.vector.tensor_tensor(out=sm[:], in0=sm[:],
                                                in1=apv[:, :, C],
                                                op=ALU.mult)
                        nc.vector.tensor_tensor(
                            out=ex[:], in0=ex[:],
                            in1=sm[:].to_broadcast([128, 8, C]),
                            op=ALU.mult)
                        msg = sbp.tile([128, 8, CN], BF16, tag="msg")
                        nc.vector.tensor_tensor(
                            out=msg[:].rearrange("q b (c f) -> q b c f", f=FN),
                            in0=gn[:, :, 0:CN].rearrange(
                                "q b (c f) -> q b c f", f=FN),
                            in1=ex[:].to_broadcast([128, 8, C, FN]),
                            op=ALU.mult)
                        for k in range(8):
                            ci = j * 8 + k
                            t = int(chunks[ph][ci])
                            pr = t % 2
                            if first[ph][ci] and (pr == 0 or t == 0
                                                  or sps1 is None):
                                sps1 = spp.tile([128, 256], F32, tag="sps1",
                                                space="PSUM")
                                sps2 = spp.tile([FN, 256], F32, tag="sps2",
                                                space="PSUM")
                            psl = slice(pr * 128, (pr + 1) * 128)
                            nc.tensor.matmul(
                                out=sps1[:, psl], lhsT=msg[:, k, 0:128],
                                rhs=Ost[:, k, :],
                                start=bool(first[ph][ci]),
                                stop=bool(last[ph][ci]))
                            nc.tensor.matmul(
                                out=sps2[:, psl], lhsT=msg[:, k, 128:160],
                                rhs=Ost[:, k, :],
                                start=bool(first[ph][ci]),
                                stop=bool(last[ph][ci]))
                            if last[ph][ci] and (pr == 1 or t == NMT - 1):
                                t0c = t - pr
                                w = (pr + 1) * 128
                                tsl = slice(t0c * 128, t0c * 128 + w)
                                nc.vector.tensor_copy(
                                    out=sfm01[:, tsl],
                                    in_=sps1[0:2 * FN, 0:w])
                                nc.vector.tensor_copy(
                                    out=sfm23[:, tsl],
                                    in_=sps1[2 * FN:, 0:w])
                                nc.vector.tensor_copy(out=sfm4[:, tsl],
                                                      in_=sps2[:, 0:w])
                                sps1 = sps2 = None
                    # ---- down MLP (class pairs) ----
                    od = out_d[p, h]  # [C, FP, NHP]
                    for ch0 in range(0, NHP, CHW):
                        cw = min(CHW, NHP - ch0)
                        csl = slice(ch0, ch0 + cw)
                        for gi in range(2):
                            hp = mpp2.tile([128, CHW], F32, tag="hp",
                                           space="PSUM")
                            nc.tensor.matmul(out=hp[:, :cw],
                                             lhsT=wd1x_t[p][gi][:],
                                             rhs=(ftx01 if gi == 0
                                                  else ftx23)[:, csl],
                                             start=True, stop=False)
                            nc.tensor.matmul(out=hp[:, :cw],
                                             lhsT=wd1s_t[p][gi][:],
                                             rhs=(sfm01 if gi == 0
                                                  else sfm23)[:, csl],
                                             start=False, stop=True)
                            ht = obp.tile([128, CHW], BF16, tag="ht")
                            nc.scalar.activation(ht[:, :cw], hp[:, :cw], TANH,
                                                 bias=bd1p_t[p][gi][:])
                            op_ = mpp2.tile([128, CHW], F32, tag="op",
                                            space="PSUM")
                            nc.tensor.matmul(out=op_[:, :cw],
                                             lhsT=wd2p_t[p][gi][:],
                                             rhs=ht[:, :cw],
                                             start=True, stop=True)
                            ot = obp.tile([128, CHW], F32, tag="ot")
                            nc.scalar.activation(ot[:, :cw], op_[:, :cw],
                                                 TANH, bias=bd2p_t[p][gi][:])
                            nc.sync.dma_start(
                                out=od[2 * gi:2 * gi + 2]
                                .rearrange("c f n -> (c f) n")[:, csl],
                                in_=ot[:, :cw])
                        hp4f = mpp2.tile([128, CHW], F32, tag="hp",
                                         space="PSUM")
                        hp4 = hp4f[0:FP, :cw]
                        nc.tensor.matmul(out=hp4, lhsT=wd1x4_t[p][:],
                                         rhs=ftx4[:, csl],
                                         start=True, stop=False)
                        nc.tensor.matmul(out=hp4, lhsT=wd1s4_t[p][:],
                                         rhs=sfm4[:, csl],
                                         start=False, stop=True)
                        ht4f = obp.tile([128, CHW], BF16, tag="ht")
                        ht4 = ht4f[0:FP, :]
                        nc.scalar.activation(ht4[:, :cw], hp4, TANH,
                                             bias=bd1_4_t[p][:])
                        op4f = mpp2.tile([128, CHW], F32, tag="op",
                                         space="PSUM")
                        op4 = op4f[0:FP, :cw]
                        nc.tensor.matmul(out=op4, lhsT=wd2_4_t[p][:],
                                         rhs=ht4[:, :cw], start=True,
                                         stop=True)
                        ot4f = obp.tile([128, CHW], F32, tag="ot")
                        ot4 = ot4f[0:FP, :]
                        nc.scalar.activation(ot4[:, :cw], op4, TANH,
                                             bias=bd2_4_t[p][:])
                        nc.sync.dma_start(out=od[4][:, csl], in_=ot4[:, :cw])

    nc.compile()
    return nc


_CACHE = {}


def _get_compiled(inputs, cfg):
    in_maps, meta = host_prep(inputs, cfg)
    key = (meta["K_UP"], meta["NCALLD"],
           meta["chunks"].tobytes(), tuple(sorted(cfg.items())))
    if key not in _CACHE:
        _CACHE[key] = build_kernel(meta)
    return _CACHE[key], in_maps, meta


def assemble_output(results, meta):
    cfg = meta["cfg"]
    P, N, C, FP, NC = (cfg[k] for k in ("P", "N", "C", "FP", "NC"))
    NH = meta["NH"]
    arr = np.stack([np.asarray(results[k]["outT"])[:, :, :, :, :NH]
                    for k in range(NC)])
    out = arr.transpose(1, 0, 2, 5, 3, 4).reshape(P, N, C, FP)
    return np.ascontiguousarray(out)


def kernel(**inputs):
    from concourse.bass_utils import run_bass_kernel_spmd
    cfg = CFG_FULL
    nc, in_maps, meta = _get_compiled(inputs, cfg)
    res = run_bass_kernel_spmd(nc, in_maps, list(range(cfg["NC"])))
    return assemble_output(res.results, meta)
